# revision 29
# baseline (speedup 1.0000x reference)
"""Trainium2 Bass kernel for nn_MoEDetector (moe_routing).

Strategy: data-parallel over batch B=8 -> one batch per NeuronCore, plus
top-1 expert bucketing so the syn/sem groups run ~3/8 of the dense work.

Host side (cheap, exact):
  - router logits/probs/argmax + group coefficients in fp32 numpy
    (top-2 logit gaps are ~1e-4 while fp32 sum-order noise is ~1e-6, so
    the argmax always matches the jax reference)
  - tokens sorted by syn expert (perm applied to hs, adj rows+cols);
    second sort by sem expert gives hs_sem
  - per-expert compile-time column WINDOWS [re_e, re_e+cap_e*128) that
    cover the bucket on every core (offsets differ per core; the window
    union is compile-time, per-core masking via zeroed coefficients)
  - adjacency degree-normalized + transposed, hs transposed, both bf16
  - final per-group cls outputs are unpermuted and summed on host

Device program (shared by all 8 cores; per-core tensor CONTENT differs):
  - GCN1 -> agg1(relu) -> GCN2 -> agg2(relu) -> +hs residual -> LayerNorm
    (affine folded into syn weights), matmuls bf16, accumulation fp32
  - experts run TRANSPOSED: out_T[d,tok] = W^T @ x_T, so the gelu output
    lands pre-transposed for the cls projection and the per-token
    coefficient factors out of the d-contraction -> applied after cls as
    a per-partition scalar on the [slots,2] result
  - groups: syn (3 windows on sharedT), len (all tokens on hsT),
    sem (3 windows on hs_semT); each -> fusedT bf16 -> cls -> out rows
"""

import numpy as np
import ml_dtypes
from contextlib import ExitStack

B, S, H = 8, 1024, 1536
THRESHOLD = 128
NEG = -1e9
P = 128
ST = S // P          # 8 token tiles
KT = H // P          # 12 h tiles
TT = S // P          # 8 t tiles
NCH = 512            # matmul moving free-dim chunk
NN = H // NCH        # 3 chunks of H
EPS = 1e-5
SPAD = 384           # pad tail so expert windows may overrun S

_BF16 = ml_dtypes.bfloat16

_prog_cache = {}


# ---------------------------------------------------------------- host math
def _route_host(hs, rw, rb, seq_lengths):
    """fp32 numpy replication of the reference router."""
    logits = (hs.reshape(-1, H).astype(np.float32) @ rw).reshape(B, S, 8) + rb
    is_short = (np.asarray(seq_lengths) <= THRESHOLD)
    lg = logits.copy()
    lg[..., 3] = np.where(is_short[:, None], logits[..., 3], NEG)
    lg[..., 4] = np.where(is_short[:, None], NEG, logits[..., 4])
    m = lg.max(-1, keepdims=True)
    e = np.exp((lg - m).astype(np.float32))
    probs = (e / e.sum(-1, keepdims=True)).astype(np.float32)
    syn_p = probs[..., 0:3].max(-1)
    syn_i = probs[..., 0:3].argmax(-1)
    len_p = probs[..., 3:5].max(-1)
    sem_p = probs[..., 5:8].max(-1)
    sem_i = probs[..., 5:8].argmax(-1)
    den = syn_p + len_p + sem_p
    return ((syn_p / den).astype(np.float32), syn_i,
            (len_p / den).astype(np.float32),
            (sem_p / den).astype(np.float32), sem_i, is_short)


def _windows(idx_sorted):
    """idx_sorted: [B, S] expert index per token, sorted ascending per row.
    Returns (re, caps): compile-time window starts and tile capacities
    covering bucket e on every core."""
    re, caps = [], []
    for e in range(3):
        starts = (idx_sorted < e).sum(axis=1)      # bucket start per core
        ends = (idx_sorted <= e).sum(axis=1)       # bucket end per core
        r = int(starts.min())
        hi = int(ends.max())
        re.append(r)
        caps.append(max(0, -(-(hi - r) // P)))     # ceil
    return tuple(re), tuple(caps)


# ---------------------------------------------------------------- device IR
def _build_program(geom):
    """geom = (re_syn, caps_syn, re_sem, caps_sem, synb_nz, lenb_nz, semb_nz)"""
    import concourse.bass as bass
    import concourse.tile as tile
    from concourse import bacc, masks, mybir

    re_syn, caps_syn, re_sem, caps_sem, synb_nz, lenb_nz, semb_nz = geom
    C_syn = sum(caps_syn)
    C_sem = sum(caps_sem)
    NT = C_syn + ST + C_sem                        # output tiles total
    spad_syn = max([0] + [re_syn[e] + caps_syn[e] * P - S for e in range(3)])
    spad_sem = max([0] + [re_sem[e] + caps_sem[e] * P - S for e in range(3)])
    f32 = mybir.dt.float32
    bf16 = mybir.dt.bfloat16
    AF = mybir.ActivationFunctionType
    ALU = mybir.AluOpType
    AX = mybir.AxisListType
    ts = bass.ts

    nc = bacc.Bacc("TRN2", target_bir_lowering=False, debug=False)

    # ---- DRAM I/O ----
    hsT_d = nc.dram_tensor("hsT", [H, S], bf16, kind="ExternalInput").ap()
    adjT_d = nc.dram_tensor("adjT", [S, S], bf16, kind="ExternalInput").ap()
    hs_d = nc.dram_tensor("hs", [S, H], f32, kind="ExternalInput").ap()
    semT_d = nc.dram_tensor("semT", [H, S], bf16, kind="ExternalInput").ap()
    wg1_d = nc.dram_tensor("wg1", [H, H], bf16, kind="ExternalInput").ap()
    wg2_d = nc.dram_tensor("wg2", [H, H], bf16, kind="ExternalInput").ap()
    wsyn_d = nc.dram_tensor("wsyn", [3, H, H], bf16, kind="ExternalInput").ap()
    wlen_d = nc.dram_tensor("wlen", [H, H], bf16, kind="ExternalInput").ap()
    wsem_d = nc.dram_tensor("wsem", [3, H, H], bf16, kind="ExternalInput").ap()
    wcls_d = nc.dram_tensor("wcls", [H, 2], bf16, kind="ExternalInput").ap()
    csyn_d = nc.dram_tensor("csyn", [max(C_syn, 1) * P], f32, kind="ExternalInput").ap()
    clen_d = nc.dram_tensor("clen", [S], f32, kind="ExternalInput").ap()
    csem_d = nc.dram_tensor("csem", [max(C_sem, 1) * P], f32, kind="ExternalInput").ap()
    bsyn_d = nc.dram_tensor("bsyn", [3, H], f32, kind="ExternalInput").ap() if synb_nz else None
    blen_d = nc.dram_tensor("blen", [1, H], f32, kind="ExternalInput").ap() if lenb_nz else None
    bsem_d = nc.dram_tensor("bsem", [3, H], f32, kind="ExternalInput").ap() if semb_nz else None
    out_d = nc.dram_tensor("out", [NT * P, 2], f32, kind="ExternalOutput").ap()

    hsT_r = hsT_d.rearrange("(k p) s -> p k s", p=P)
    adjT_r = adjT_d.rearrange("(t p) s -> p t s", p=P)
    hs_r = hs_d.rearrange("(a p) h -> p a h", p=P)
    semT_r = semT_d.rearrange("(k p) s -> p k s", p=P)
    wcls_r = wcls_d.rearrange("(k p) c -> p k c", p=P)
    csyn_r = csyn_d.rearrange("(a p) -> p a", p=P)
    clen_r = clen_d.rearrange("(a p) -> p a", p=P)
    csem_r = csem_d.rearrange("(a p) -> p a", p=P)
    out_r = out_d.rearrange("(a p) c -> p a c", p=P)

    with tile.TileContext(nc) as tc, ExitStack() as ctx:
        # ---- pools ----
        const = ctx.enter_context(tc.tile_pool(name="const", bufs=1))
        hsempool = ctx.enter_context(tc.tile_pool(name="hsempool", bufs=1))
        adjpool = ctx.enter_context(tc.tile_pool(name="adjpool", bufs=1))
        suppool = ctx.enter_context(tc.tile_pool(name="suppool", bufs=1))
        bigpool = ctx.enter_context(tc.tile_pool(name="bigpool", bufs=1))
        wpool = ctx.enter_context(tc.tile_pool(name="wpool", bufs=24))
        small = ctx.enter_context(tc.tile_pool(name="small", bufs=2))
        transb = ctx.enter_context(tc.tile_pool(name="transb", bufs=2))
        transf = ctx.enter_context(tc.tile_pool(name="transf", bufs=2))
        rowf32 = ctx.enter_context(tc.tile_pool(name="rowf32", bufs=1))
        chunkp = ctx.enter_context(tc.tile_pool(name="chunkp", bufs=3))
        acc = ctx.enter_context(tc.tile_pool(name="acc", bufs=5, space="PSUM"))
        tp = ctx.enter_context(tc.tile_pool(name="tp", bufs=2, space="PSUM"))
        clsps = ctx.enter_context(tc.tile_pool(name="clsps", bufs=1, space="PSUM"))

        # ---- constants / small inputs ----
        id_bf = const.tile([P, P], bf16, tag="idb")
        masks.make_identity(nc, id_bf[:])
        eps_t = const.tile([P, 1], f32, tag="eps")
        nc.vector.memset(eps_t[:], EPS)
        wcls_sb = const.tile([P, KT, 2], bf16, tag="wcls")
        nc.gpsimd.dma_start(wcls_sb[:], wcls_r)
        csyn_t = const.tile([P, max(C_syn, 1)], f32, tag="csyn")
        nc.gpsimd.dma_start(csyn_t[:], csyn_r)
        clen_t = const.tile([P, ST], f32, tag="clen")
        nc.gpsimd.dma_start(clen_t[:], clen_r)
        csem_t = const.tile([P, max(C_sem, 1)], f32, tag="csem")
        nc.gpsimd.dma_start(csem_t[:], csem_r)
        ones_row = None
        if synb_nz or lenb_nz or semb_nz:
            ones_row = const.tile([1, NCH], f32, tag="ones")
            nc.vector.memset(ones_row[:], 1.0)

        def bias_row(dram_ap, tag):
            t = const.tile([1, H], f32, tag=tag)
            nc.gpsimd.dma_start(t[:], dram_ap)
            return t

        bsyn_sb = ([bias_row(bsyn_d[e : e + 1, :], f"bsyn{e}") for e in range(3)]
                   if bsyn_d is not None else [None] * 3)
        blen_sb = bias_row(blen_d, "blen") if blen_d is not None else None
        bsem_sb = ([bias_row(bsem_d[e : e + 1, :], f"bsem{e}") for e in range(3)]
                   if bsem_d is not None else [None] * 3)

        # ---- big persistent SBUF tensors ----
        # hsT and (later) semT share one pool slot: hsT is dead after GCN1,
        # semT is only needed for the final sem expert group.
        hsT = hsempool.tile([P, KT, S + spad_sem], bf16, tag="hsem")
        bigT = bigpool.tile([P, KT, S + spad_syn], bf16, tag="bigT")
        for k in range(KT):
            if spad_syn:
                nc.vector.memset(bigT[:, k, S:], 0.0)
        sup = suppool.tile([P, ST, H], bf16, tag="sup")
        out_sb = small.tile([P, NT, 2], f32, tag="outsb")

        def load_wtiles(wdram):
            tiles = []
            for k in range(KT):
                wt = wpool.tile([P, H], bf16, tag="w")
                nc.sync.dma_start(wt[:], wdram[ts(k, P), :])
                tiles.append(wt)
            return tiles

        def transpose_into(dstT, src_of_k, m, ident):
            for k in range(KT):
                pt = tp.tile([P, P], ident.dtype, tag="tp")
                nc.tensor.transpose(pt[:], src_of_k(k), ident[:])
                nc.any.tensor_copy(dstT[:, k, ts(m, P)], pt[:])

        # ---- expert group helper (transposed form, fused cls) ----
        # One "window" = (rhsT, rhs_col0, ntiles, weight dram, bias, coef tile
        # column offset, out_sb tile offset). Emits, per (d, chunk):
        # expert matmuls -> [pipelined cls of the previous chunk] -> gelu.
        # cls partials accumulate over d into an SBUF accumulator; the
        # per-token coefficient is applied after cls (it factors out of the
        # d-contraction), so padding/masked slots simply get multiplied by 0.
        def expert_window(wdram, rhsT, rhs_col0, ntiles, bias_sb, coef_t,
                          coef_off, out_tile0, dma_with=None, stream_first=0):
            wt = []
            for k in range(KT):
                if dma_with is not None:
                    dma_with(k)
                wk = wpool.tile([P, H], bf16, tag="w")
                nc.sync.dma_start(wk[:], wdram[ts(k, P), :])
                wt.append(wk)
            width = ntiles * P
            nch = [(c0, min(NCH, width - c0)) for c0 in range(0, width, NCH)]
            groups = [(d, c0, w) for d in range(KT) for (c0, w) in nch]
            clsacc = small.tile([P, ntiles, 2], f32, tag="clsacc")
            nc.vector.memset(clsacc[:], 0.0)
            pending = None

            def emit_pending(p):
                ch, pc0, pw, pd = p
                nj = pw // P
                psd = clsps.tile([P, nj, 2], f32, tag="cls")
                for jj in range(nj):
                    nc.tensor.matmul(psd[:, jj, :], ch[:, ts(jj, P)],
                                     wcls_sb[:, pd, :],
                                     start=True, stop=True)
                j0 = pc0 // P
                nc.vector.tensor_add(clsacc[:, j0 : j0 + nj, :],
                                     clsacc[:, j0 : j0 + nj, :], psd[:])

            def finish_group(ps, d, c0, w):
                nonlocal pending
                if bias_sb is not None:
                    nc.tensor.matmul(ps[:, :w], bias_sb[:, ts(d, P)],
                                     ones_row[:, :w], start=False, stop=True)
                if pending is not None:
                    emit_pending(pending)
                ch = chunkp.tile([P, NCH], bf16, tag="ch")
                nc.scalar.activation(ch[:, :w], ps[:, :w], AF.Gelu)
                pending = (ch, c0, w, d)

            gi = 0
            if stream_first > 1:
                # k-outer over the first few groups so matmuls overlap the
                # initial weight/activation DMA stream tile-by-tile
                blk = groups[:stream_first]
                pss = []
                for _ in blk:
                    psk = acc.tile([P, NCH], f32, tag="acc")
                    pss.append(psk)
                for k in range(KT):
                    for ps, (d, c0, w) in zip(pss, blk):
                        last = (k == KT - 1) and (bias_sb is None)
                        nc.tensor.matmul(ps[:, :w], wt[k][:, ts(d, P)],
                                         rhsT[:, k, rhs_col0 + c0 : rhs_col0 + c0 + w],
                                         start=(k == 0), stop=last)
                for ps, (d, c0, w) in zip(pss, blk):
                    finish_group(ps, d, c0, w)
                gi = stream_first
            for d, c0, w in groups[gi:]:
                ps = acc.tile([P, NCH], f32, tag="acc")
                for k in range(KT):
                    last = (k == KT - 1) and (bias_sb is None)
                    nc.tensor.matmul(ps[:, :w], wt[k][:, ts(d, P)],
                                     rhsT[:, k, rhs_col0 + c0 : rhs_col0 + c0 + w],
                                     start=(k == 0), stop=last)
                finish_group(ps, d, c0, w)
            emit_pending(pending)
            pending = None
            for j in range(ntiles):
                nc.vector.tensor_scalar_mul(out_sb[:, out_tile0 + j, :],
                                            clsacc[:, j, :],
                                            coef_t[:, coef_off + j : coef_off + j + 1])

        # ---- len expert first (frees hsT for semT reuse) ----
        def dma_hsT_k(k):
            nc.sync.dma_start(hsT[:, k, :S], hsT_r[:, k, :])

        expert_window(wlen_d, hsT, 0, ST, blen_sb, clen_t, 0, C_syn,
                      dma_with=dma_hsT_k, stream_first=5)
        nc.gpsimd.dma_start(out_r[:, C_syn : C_syn + ST, :],
                            out_sb[:, C_syn : C_syn + ST, :])

        # ---- GCN layer 1: sup1 = hs @ Wg1 (k-outer: start on first tiles) ----
        w_g1 = load_wtiles(wg1_d)
        for n in range(NN):
            for mb in (range(0, 3), range(3, 6), range(6, 8)):
                pss = {}
                for m in mb:
                    psk = acc.tile([P, NCH], f32, tag="acc")
                    pss[m] = psk
                for k in range(KT):
                    for m in mb:
                        nc.tensor.matmul(pss[m][:], hsT[:, k, ts(m, P)],
                                         w_g1[k][:, ts(n, NCH)],
                                         start=(k == 0), stop=(k == KT - 1))
                for m in mb:
                    nc.any.tensor_copy(sup[:, m, ts(n, NCH)], pss[m][:])

        # ---- agg1 (transposed): x1T[d,tok] = relu(sup1^T @ A_hat^T) ----
        # lhsT = sup1 tile (contraction over source tokens on partitions),
        # rhs = adjT tile; the agg matmul itself produces x1T -> no transposes
        adjT = adjpool.tile([P, TT, S], bf16, tag="adjT")
        nc.sync.dma_start(adjT[:], adjT_r)
        # semT replaces hsT in the shared slot (loads during GCN phases)
        semT = hsempool.tile([P, KT, S + spad_sem], bf16, tag="hsem")
        for k in range(KT):
            if spad_sem:
                nc.vector.memset(semT[:, k, S:], 0.0)
        nc.sync.dma_start(semT[:, :, :S], semT_r)
        w_g2 = load_wtiles(wg2_d)
        for d in range(KT):
            for c in range(S // NCH):
                ps = acc.tile([P, NCH], f32, tag="acc")
                for t in range(TT):
                    nc.tensor.matmul(ps[:], sup[:, t, ts(d, P)],
                                     adjT[:, t, ts(c, NCH)],
                                     start=(t == 0), stop=(t == TT - 1))
                nc.scalar.activation(bigT[:, d, ts(c, NCH)], ps[:], AF.Relu)

        # ---- GCN layer 2: sup2 = x1 @ Wg2 ----
        for m in range(ST):
            for n in range(NN):
                ps = acc.tile([P, NCH], f32, tag="acc")
                for k in range(KT):
                    nc.tensor.matmul(ps[:], bigT[:, k, ts(m, P)],
                                     w_g2[k][:, ts(n, NCH)],
                                     start=(k == 0), stop=(k == KT - 1))
                nc.any.tensor_copy(sup[:, m, ts(n, NCH)], ps[:])

        # ---- agg2 + residual + LayerNorm -> sharedT (into bigT) ----
        # Transposes run one m behind so the LN chain (DVE/Act) overlaps the
        # next tile's agg matmuls instead of stalling PE.
        prev_xb = None
        for m in range(ST):
            hsm = rowf32.tile([P, H], f32, tag="hsm")
            nc.sync.dma_start(hsm[:], hs_r[:, m, :])
            x2row = transf.tile([P, H], f32, tag="x2row")
            for n in range(NN):
                ps = acc.tile([P, NCH], f32, tag="acc")
                for t in range(TT):
                    nc.tensor.matmul(ps[:], adjT[:, t, ts(m, P)],
                                     sup[:, t, ts(n, NCH)],
                                     start=(t == 0), stop=(t == TT - 1))
                nc.scalar.activation(x2row[:, ts(n, NCH)], ps[:], AF.Relu)
                if n == 0 and prev_xb is not None:
                    xb, pm = prev_xb
                    transpose_into(bigT, lambda k: xb[:, ts(k, P)], pm, id_bf)
            nc.vector.tensor_add(x2row[:], x2row[:], hsm[:])
            stats = small.tile([P, NN, 6], f32, tag="stats")
            for c in range(NN):
                nc.vector.bn_stats(stats[:, c, :], x2row[:, ts(c, NCH)])
            mv = small.tile([P, 2], f32, tag="mv")
            nc.vector.bn_aggr(mv[:], stats[:])
            rstd = small.tile([P, 1], f32, tag="rstd")
            nc.scalar.activation(rstd[:], mv[:, 1:2], AF.Sqrt, bias=eps_t[:])
            nc.vector.reciprocal(rstd[:], rstd[:])
            xb = transb.tile([P, H], bf16, tag="xrow")
            nc.vector.tensor_scalar(out=xb[:], in0=x2row[:],
                                    scalar1=mv[:, 0:1], scalar2=rstd[:],
                                    op0=ALU.subtract, op1=ALU.mult)
            prev_xb = (xb, m)
        xb, pm = prev_xb
        transpose_into(bigT, lambda k: xb[:, ts(k, P)], pm, id_bf)

        # ---- syn experts on sharedT (bigT) ----
        fo = 0
        for e in range(3):
            if caps_syn[e]:
                expert_window(wsyn_d[e], bigT, re_syn[e], caps_syn[e],
                              bsyn_sb[e], csyn_t, fo, fo)
                fo += caps_syn[e]
        if C_syn:
            nc.gpsimd.dma_start(out_r[:, 0:C_syn, :], out_sb[:, 0:C_syn, :])

        # ---- sem experts on semT ----
        fo = 0
        for e in range(3):
            if caps_sem[e]:
                expert_window(wsem_d[e], semT, re_sem[e], caps_sem[e],
                              bsem_sb[e], csem_t, fo, C_syn + ST + fo)
                fo += caps_sem[e]
                nc.gpsimd.dma_start(
                    out_r[:, C_syn + ST + fo - caps_sem[e] : C_syn + ST + fo, :],
                    out_sb[:, C_syn + ST + fo - caps_sem[e] : C_syn + ST + fo, :])

    nc.compile()
    return nc


def _get_program(geom):
    if geom not in _prog_cache:
        _prog_cache[geom] = _build_program(geom)
    return _prog_cache[geom]


# ---------------------------------------------------------------- host glue
def _prepare(inputs):
    """Compute routing, permutations, windows; build per-core in_maps and
    decode metadata. Returns (geom, in_maps, meta)."""
    hs = np.asarray(inputs["hidden_states"], dtype=np.float32)
    adj = np.asarray(inputs["adj_matrix"], dtype=np.float32)
    seq_lengths = np.asarray(inputs["seq_lengths"])
    router_w = np.asarray(inputs["router_w"], dtype=np.float32)
    router_b = np.asarray(inputs["router_b"], dtype=np.float32)
    gcn1_w = np.asarray(inputs["gcn1_w"], dtype=np.float32)
    gcn2_w = np.asarray(inputs["gcn2_w"], dtype=np.float32)
    ln_g = np.asarray(inputs["ln_g"], dtype=np.float32)
    ln_b = np.asarray(inputs["ln_b"], dtype=np.float32)
    syn_w = np.asarray(inputs["syn_w"], dtype=np.float32)
    syn_b = np.asarray(inputs["syn_b"], dtype=np.float32)
    len_short_w = np.asarray(inputs["len_short_w"], dtype=np.float32)
    len_short_b = np.asarray(inputs["len_short_b"], dtype=np.float32)
    len_long_w = np.asarray(inputs["len_long_w"], dtype=np.float32)
    len_long_b = np.asarray(inputs["len_long_b"], dtype=np.float32)
    sem_w = np.asarray(inputs["sem_w"], dtype=np.float32)
    sem_b = np.asarray(inputs["sem_b"], dtype=np.float32)
    cls_w = np.asarray(inputs["cls_w"], dtype=np.float32)
    cls_b = np.asarray(inputs["cls_b"], dtype=np.float32)

    c_syn, syn_i, c_len, c_sem, sem_i, is_short = _route_host(
        hs, router_w, router_b, seq_lengths)

    # fold LN affine into syn weights: LN_plain(x) @ (g*W) + (b@W + bias)
    syn_w_f = (ln_g[None, :, None] * syn_w).astype(np.float32)
    syn_b_f = (syn_b + np.einsum("h,ehd->ed", ln_b, syn_w)).astype(np.float32)

    perm = np.argsort(syn_i, axis=1, kind="stable")          # [B,S]
    syn_i_p = np.take_along_axis(syn_i, perm, axis=1)
    sem_i_p = np.take_along_axis(sem_i, perm, axis=1)
    sem_perm = np.argsort(sem_i_p, axis=1, kind="stable")    # syn-order -> sem-order
    sem_i_s = np.take_along_axis(sem_i_p, sem_perm, axis=1)

    re_syn, caps_syn = _windows(syn_i_p)
    re_sem, caps_sem = _windows(sem_i_s)
    C_syn, C_sem = sum(caps_syn), sum(caps_sem)

    synb_nz = bool(np.any(syn_b_f != 0))
    lenb_nz = bool(np.any(len_short_b != 0) or np.any(len_long_b != 0))
    semb_nz = bool(np.any(sem_b != 0))
    geom = (re_syn, caps_syn, re_sem, caps_sem, synb_nz, lenb_nz, semb_nz)

    wg1 = np.ascontiguousarray(gcn1_w.astype(_BF16))
    wg2 = np.ascontiguousarray(gcn2_w.astype(_BF16))
    wsyn = np.ascontiguousarray(syn_w_f.astype(_BF16))
    wlen_s = np.ascontiguousarray(len_short_w.astype(_BF16))
    wlen_l = np.ascontiguousarray(len_long_w.astype(_BF16))
    wsem = np.ascontiguousarray(sem_w.astype(_BF16))
    wcls = np.ascontiguousarray(cls_w.astype(_BF16))

    def win_coef(cvec, idx_sorted_row, re, caps, grp):
        """Per-window masked coefficients, zero-padded to caps*P."""
        out = np.zeros(max(sum(caps), 1) * P, np.float32)
        off = 0
        for e in range(3):
            w = caps[e] * P
            lo = re[e]
            hi = min(S, lo + w)
            seg = np.where(idx_sorted_row[lo:hi] == e, cvec[lo:hi], 0.0)
            out[off : off + (hi - lo)] = seg
            off += w
        return out

    in_maps = []
    meta = []
    for b in range(B):
        p = perm[b]
        sp = sem_perm[b]
        hs_p = hs[b][p]
        adj_p = adj[b][p][:, p]
        deg = np.maximum(adj_p.sum(axis=1, dtype=np.float32), 1e-9)
        adj_n = adj_p / deg[:, None]
        hs_sem = hs_p[sp]
        c_syn_p = c_syn[b][p]
        c_len_p = c_len[b][p]
        c_sem_s = c_sem[b][p][sp]

        m = {
            "hsT": np.ascontiguousarray(hs_p.T.astype(_BF16)),
            "adjT": np.ascontiguousarray(adj_n.T.astype(_BF16)),
            "hs": np.ascontiguousarray(hs_p),
            "semT": np.ascontiguousarray(hs_sem.T.astype(_BF16)),
            "wg1": wg1, "wg2": wg2, "wsyn": wsyn,
            "wlen": wlen_s if is_short[b] else wlen_l,
            "wsem": wsem, "wcls": wcls,
            "csyn": win_coef(c_syn_p, syn_i_p[b], re_syn, caps_syn, "syn"),
            "clen": np.ascontiguousarray(c_len_p),
            "csem": win_coef(c_sem_s, sem_i_s[b], re_sem, caps_sem, "sem"),
        }
        if synb_nz:
            m["bsyn"] = syn_b_f
        if lenb_nz:
            m["blen"] = (len_short_b if is_short[b]
                         else len_long_b).reshape(1, H).astype(np.float32)
        if semb_nz:
            m["bsem"] = sem_b.astype(np.float32)
        in_maps.append(m)
        meta.append((p, sp))

    return geom, in_maps, meta, cls_b


def _decode(out_rows, geom, meta_b, cls_b):
    """out_rows: [NT*P, 2] device output for one core -> [S,2] original order."""
    re_syn, caps_syn, re_sem, caps_sem = geom[0], geom[1], geom[2], geom[3]
    C_syn, C_sem = sum(caps_syn), sum(caps_sem)
    p, sp = meta_b
    acc_syn = np.zeros((S, 2), np.float32)   # syn-order accumulation
    off = 0
    for e in range(3):
        w = caps_syn[e] * P
        lo = re_syn[e]
        hi = min(S, lo + w)
        acc_syn[lo:hi] += out_rows[off : off + (hi - lo)]
        off += w
    acc_syn += out_rows[C_syn * P : C_syn * P + S]          # len group
    acc_sem = np.zeros((S, 2), np.float32)   # sem-order
    off = (C_syn + ST) * P
    for e in range(3):
        w = caps_sem[e] * P
        lo = re_sem[e]
        hi = min(S, lo + w)
        acc_sem[lo:hi] += out_rows[off : off + (hi - lo)]
        off += w
    acc_syn[sp] += acc_sem
    res = np.empty((S, 2), np.float32)
    res[p] = acc_syn
    return res + cls_b


def kernel(**inputs):
    from concourse import bass_utils

    geom, in_maps, meta, cls_b = _prepare(inputs)
    nc = _get_program(geom)

    try:
        res = bass_utils.run_bass_kernel_spmd(nc, in_maps, core_ids=list(range(B)))
    except Exception:
        # transient device wedge (NRT_EXEC_UNIT_UNRECOVERABLE) clears on retry
        res = bass_utils.run_bass_kernel_spmd(nc, in_maps, core_ids=list(range(B)))
    globals()["_last_results"] = res
    out = np.stack([_decode(np.asarray(res.results[b]["out"], np.float32),
                            geom, meta[b], cls_b)
                    for b in range(B)]).astype(np.float32)
    return out


# revision 39
# speedup vs baseline: 1.0250x; 1.0250x over previous
"""Trainium2 Bass kernel for nn_MoEDetector (moe_routing).

Strategy: data-parallel over batch B=8 -> one batch per NeuronCore, plus
top-1 expert bucketing so the syn/sem groups run ~3/8 of the dense work.

Host side (cheap, exact):
  - router logits/probs/argmax + group coefficients in fp32 numpy
    (top-2 logit gaps are ~1e-4 while fp32 sum-order noise is ~1e-6, so
    the argmax always matches the jax reference)
  - tokens sorted by syn expert (perm applied to hs, adj rows+cols);
    second sort by sem expert gives hs_sem
  - per-expert compile-time column WINDOWS [re_e, re_e+cap_e*128) that
    cover the bucket on every core (offsets differ per core; the window
    union is compile-time, per-core masking via zeroed coefficients)
  - adjacency degree-normalized + transposed, hs transposed, both bf16
  - final per-group cls outputs are unpermuted and summed on host

Device program (shared by all 8 cores; per-core tensor CONTENT differs):
  - GCN1 -> agg1(relu) -> GCN2 -> agg2(relu) -> +hs residual -> LayerNorm
    (affine folded into syn weights), matmuls bf16, accumulation fp32
  - experts run TRANSPOSED: out_T[d,tok] = W^T @ x_T, so the gelu output
    lands pre-transposed for the cls projection and the per-token
    coefficient factors out of the d-contraction -> applied after cls as
    a per-partition scalar on the [slots,2] result
  - groups: syn (3 windows on sharedT), len (all tokens on hsT),
    sem (3 windows on hs_semT); each -> fusedT bf16 -> cls -> out rows
"""

import numpy as np
import ml_dtypes
from contextlib import ExitStack

B, S, H = 8, 1024, 1536
THRESHOLD = 128
NEG = -1e9
P = 128
ST = S // P          # 8 token tiles
KT = H // P          # 12 h tiles
TT = S // P          # 8 t tiles
NCH = 512            # matmul moving free-dim chunk
NN = H // NCH        # 3 chunks of H
EPS = 1e-5
SPAD = 384           # pad tail so expert windows may overrun S

_BF16 = ml_dtypes.bfloat16

_prog_cache = {}


# ---------------------------------------------------------------- host math
def _route_host(hs, rw, rb, seq_lengths):
    """fp32 numpy replication of the reference router."""
    logits = (hs.reshape(-1, H).astype(np.float32) @ rw).reshape(B, S, 8) + rb
    is_short = (np.asarray(seq_lengths) <= THRESHOLD)
    lg = logits.copy()
    lg[..., 3] = np.where(is_short[:, None], logits[..., 3], NEG)
    lg[..., 4] = np.where(is_short[:, None], NEG, logits[..., 4])
    m = lg.max(-1, keepdims=True)
    e = np.exp((lg - m).astype(np.float32))
    probs = (e / e.sum(-1, keepdims=True)).astype(np.float32)
    syn_p = probs[..., 0:3].max(-1)
    syn_i = probs[..., 0:3].argmax(-1)
    len_p = probs[..., 3:5].max(-1)
    sem_p = probs[..., 5:8].max(-1)
    sem_i = probs[..., 5:8].argmax(-1)
    den = syn_p + len_p + sem_p
    return ((syn_p / den).astype(np.float32), syn_i,
            (len_p / den).astype(np.float32),
            (sem_p / den).astype(np.float32), sem_i, is_short)


def _windows(idx_sorted):
    """idx_sorted: [B, S] expert index per token, sorted ascending per row.
    Returns (re, caps, spans): compile-time window starts, tile capacities
    (output layout), and exact column spans (matmul width) covering bucket
    e on every core."""
    re, caps, spans = [], [], []
    for e in range(3):
        starts = (idx_sorted < e).sum(axis=1)      # bucket start per core
        ends = (idx_sorted <= e).sum(axis=1)       # bucket end per core
        r = int(starts.min())
        hi = int(ends.max())
        span = max(0, hi - r)
        re.append(r)
        spans.append(span)
        caps.append(-(-span // P))                 # ceil
    return tuple(re), tuple(caps), tuple(spans)


# ---------------------------------------------------------------- device IR
def _build_program(geom):
    """geom = (re_syn, caps_syn, spans_syn, re_sem, caps_sem, spans_sem,
    synb_nz, lenb_nz, semb_nz)"""
    import concourse.bass as bass
    import concourse.tile as tile
    from concourse import bacc, masks, mybir

    (re_syn, caps_syn, spans_syn, re_sem, caps_sem, spans_sem,
     synb_nz, lenb_nz, semb_nz) = geom
    C_syn = sum(caps_syn)
    C_sem = sum(caps_sem)
    NT = C_syn + ST + C_sem                        # output tiles total
    spad_syn = max([0] + [re_syn[e] + spans_syn[e] - S for e in range(3)])
    spad_sem = max([0] + [re_sem[e] + spans_sem[e] - S for e in range(3)])
    f32 = mybir.dt.float32
    bf16 = mybir.dt.bfloat16
    AF = mybir.ActivationFunctionType
    ALU = mybir.AluOpType
    AX = mybir.AxisListType
    ts = bass.ts

    nc = bacc.Bacc("TRN2", target_bir_lowering=False, debug=False)

    # ---- DRAM I/O ----
    hsT_d = nc.dram_tensor("hsT", [H, S], bf16, kind="ExternalInput").ap()
    adjT_d = nc.dram_tensor("adjT", [S, S], bf16, kind="ExternalInput").ap()
    hs_d = nc.dram_tensor("hs", [S, H], f32, kind="ExternalInput").ap()
    semT_d = nc.dram_tensor("semT", [H, S], bf16, kind="ExternalInput").ap()
    wg1_d = nc.dram_tensor("wg1", [H, H], bf16, kind="ExternalInput").ap()
    wg2_d = nc.dram_tensor("wg2", [H, H], bf16, kind="ExternalInput").ap()
    wsyn_d = nc.dram_tensor("wsyn", [3, H, H], bf16, kind="ExternalInput").ap()
    wlen_d = nc.dram_tensor("wlen", [H, H], bf16, kind="ExternalInput").ap()
    wsem_d = nc.dram_tensor("wsem", [3, H, H], bf16, kind="ExternalInput").ap()
    wcls_d = nc.dram_tensor("wcls", [H, 2], bf16, kind="ExternalInput").ap()
    csyn_d = nc.dram_tensor("csyn", [max(C_syn, 1) * P, 2], f32, kind="ExternalInput").ap()
    clen_d = nc.dram_tensor("clen", [S, 2], f32, kind="ExternalInput").ap()
    csem_d = nc.dram_tensor("csem", [max(C_sem, 1) * P, 2], f32, kind="ExternalInput").ap()
    bsyn_d = nc.dram_tensor("bsyn", [3, H], f32, kind="ExternalInput").ap() if synb_nz else None
    blen_d = nc.dram_tensor("blen", [1, H], f32, kind="ExternalInput").ap() if lenb_nz else None
    bsem_d = nc.dram_tensor("bsem", [3, H], f32, kind="ExternalInput").ap() if semb_nz else None
    out_d = nc.dram_tensor("out", [NT * P, 2], f32, kind="ExternalOutput").ap()

    hsT_r = hsT_d.rearrange("(k p) s -> p k s", p=P)
    adjT_r = adjT_d.rearrange("(t p) s -> p t s", p=P)
    hs_r = hs_d.rearrange("(a p) h -> p a h", p=P)
    semT_r = semT_d.rearrange("(k p) s -> p k s", p=P)
    wcls_r = wcls_d.rearrange("(k p) c -> p k c", p=P)
    csyn_r = csyn_d.rearrange("(a p) c -> p a c", p=P)
    clen_r = clen_d.rearrange("(a p) c -> p a c", p=P)
    csem_r = csem_d.rearrange("(a p) c -> p a c", p=P)
    out_r = out_d.rearrange("(a p) c -> p a c", p=P)

    with tile.TileContext(nc) as tc, ExitStack() as ctx:
        # ---- pools ----
        const = ctx.enter_context(tc.tile_pool(name="const", bufs=1))
        hsempool = ctx.enter_context(tc.tile_pool(name="hsempool", bufs=1))
        adjpool = ctx.enter_context(tc.tile_pool(name="adjpool", bufs=1))
        suppool = ctx.enter_context(tc.tile_pool(name="suppool", bufs=1))
        bigpool = ctx.enter_context(tc.tile_pool(name="bigpool", bufs=1))
        wpool = ctx.enter_context(tc.tile_pool(name="wpool", bufs=24))
        small = ctx.enter_context(tc.tile_pool(name="small", bufs=2))
        transb = ctx.enter_context(tc.tile_pool(name="transb", bufs=2))
        transf = ctx.enter_context(tc.tile_pool(name="transf", bufs=2))
        rowf32 = ctx.enter_context(tc.tile_pool(name="rowf32", bufs=1))
        chunkp = ctx.enter_context(tc.tile_pool(name="chunkp", bufs=3))
        sbacc = ctx.enter_context(tc.tile_pool(name="sbacc", bufs=4))
        acc = ctx.enter_context(tc.tile_pool(name="acc", bufs=5, space="PSUM"))
        tp = ctx.enter_context(tc.tile_pool(name="tp", bufs=2, space="PSUM"))
        clsps = ctx.enter_context(tc.tile_pool(name="clsps", bufs=1, space="PSUM"))

        # ---- constants / small inputs ----
        id_bf = const.tile([P, P], bf16, tag="idb")
        masks.make_identity(nc, id_bf[:])
        eps_t = const.tile([P, 1], f32, tag="eps")
        nc.vector.memset(eps_t[:], EPS)
        wcls_sb = const.tile([P, KT, 2], bf16, tag="wcls")
        nc.gpsimd.dma_start(wcls_sb[:], wcls_r)
        csyn_t = const.tile([P, max(C_syn, 1), 2], f32, tag="csyn")
        nc.gpsimd.dma_start(csyn_t[:], csyn_r)
        clen_t = const.tile([P, ST, 2], f32, tag="clen")
        nc.gpsimd.dma_start(clen_t[:], clen_r)
        csem_t = const.tile([P, max(C_sem, 1), 2], f32, tag="csem")
        nc.gpsimd.dma_start(csem_t[:], csem_r)
        ones_row = None
        if synb_nz or lenb_nz or semb_nz:
            ones_row = const.tile([1, NCH], f32, tag="ones")
            nc.vector.memset(ones_row[:], 1.0)

        def bias_row(dram_ap, tag):
            t = const.tile([1, H], f32, tag=tag)
            nc.gpsimd.dma_start(t[:], dram_ap)
            return t

        bsyn_sb = ([bias_row(bsyn_d[e : e + 1, :], f"bsyn{e}") for e in range(3)]
                   if bsyn_d is not None else [None] * 3)
        blen_sb = bias_row(blen_d, "blen") if blen_d is not None else None
        bsem_sb = ([bias_row(bsem_d[e : e + 1, :], f"bsem{e}") for e in range(3)]
                   if bsem_d is not None else [None] * 3)

        # ---- big persistent SBUF tensors ----
        # hsT and (later) semT share one pool slot: hsT is dead after GCN1,
        # semT is only needed for the final sem expert group.
        hsT = hsempool.tile([P, KT, S + spad_sem], bf16, tag="hsem")
        bigT = bigpool.tile([P, KT, S + spad_syn], bf16, tag="bigT")
        for k in range(KT):
            if spad_syn:
                nc.vector.memset(bigT[:, k, S:], 0.0)
        sup = suppool.tile([P, ST, H], bf16, tag="sup")
        out_sb = small.tile([P, NT, 2], f32, tag="outsb")

        def load_wtiles(wdram):
            tiles = []
            for k in range(KT):
                wt = wpool.tile([P, H], bf16, tag="w")
                nc.sync.dma_start(wt[:], wdram[ts(k, P), :])
                tiles.append(wt)
            return tiles

        def transpose_into(dstT, src_of_k, m, ident):
            for k in range(KT):
                pt = tp.tile([P, P], ident.dtype, tag="tp")
                nc.tensor.transpose(pt[:], src_of_k(k), ident[:])
                nc.any.tensor_copy(dstT[:, k, ts(m, P)], pt[:])

        # ---- expert group helper (transposed form, fused cls) ----
        # One "window" = (rhsT, rhs_col0, ntiles, weight dram, bias, coef tile
        # column offset, out_sb tile offset). Emits, per (d, chunk):
        # expert matmuls -> [pipelined cls of the previous chunk] -> gelu.
        # cls partials accumulate over d into an SBUF accumulator; the
        # per-token coefficient is applied after cls (it factors out of the
        # d-contraction), so padding/masked slots simply get multiplied by 0.
        def expert_window(wdram, rhsT, rhs_col0, span, ntiles, bias_sb, coef_t,
                          coef_off, out_tile0, dma_with=None, stream_first=0,
                          stream_extra=0):
            wt = []
            for k in range(KT):
                if dma_with is not None:
                    dma_with(k)
                wk = wpool.tile([P, H], bf16, tag="w")
                nc.sync.dma_start(wk[:], wdram[ts(k, P), :])
                wt.append(wk)
            nch = [(c0, min(NCH, span - c0)) for c0 in range(0, span, NCH)]
            groups = [(d, c0, w) for d in range(KT) for (c0, w) in nch]
            clsacc = small.tile([P, ntiles, 2], f32, tag="clsacc")
            nc.vector.memset(clsacc[:], 0.0)
            pending = None

            def emit_pending(p):
                # full 128-wide tiles even when the span is partial: the tail
                # columns of ch hold stale-but-finite values (the len expert
                # fills the ring with 512-wide chunks first) and their cls
                # rows get multiplied by a zero coefficient at the end.
                ch, pc0, pw, pd = p
                nj = -(-pw // P)
                j0 = pc0 // P
                psd = clsps.tile([P, nj, 2], f32, tag="cls")
                for jj in range(nj):
                    nc.tensor.matmul(psd[:, jj, :], ch[:, ts(jj, P)],
                                     wcls_sb[:, pd, :],
                                     start=True, stop=True)
                nc.vector.tensor_add(clsacc[:, j0 : j0 + nj, :],
                                     clsacc[:, j0 : j0 + nj, :], psd[:])

            def finish_group(ps, d, c0, w):
                nonlocal pending
                if bias_sb is not None:
                    nc.tensor.matmul(ps[:, :w], bias_sb[:, ts(d, P)],
                                     ones_row[:, :w], start=False, stop=True)
                if pending is not None:
                    emit_pending(pending)
                ch = chunkp.tile([P, NCH], bf16, tag="ch")
                nc.scalar.activation(ch[:, :w], ps[:, :w], AF.Gelu)
                pending = (ch, c0, w, d)

            gi = 0
            if stream_first > 1:
                # k-outer over the first groups so matmuls overlap the initial
                # weight/activation DMA stream tile-by-tile. Groups beyond the
                # PSUM capacity accumulate per-k partials into SBUF via DVE
                # (single-shot matmuls through the otherwise-idle tp banks),
                # raising PE work per arriving tile to cover the DMA window.
                blk = groups[:stream_first]
                xtr = (groups[stream_first : stream_first + stream_extra]
                       if bias_sb is None else [])
                pss = []
                for _ in blk:
                    psk = acc.tile([P, NCH], f32, tag="acc")
                    pss.append(psk)
                sacc = []
                for _ in xtr:
                    sb_t = sbacc.tile([P, NCH], f32, tag="sb")
                    sacc.append(sb_t)
                for k in range(KT):
                    for ps, (d, c0, w) in zip(pss, blk):
                        last = (k == KT - 1) and (bias_sb is None)
                        nc.tensor.matmul(ps[:, :w], wt[k][:, ts(d, P)],
                                         rhsT[:, k, rhs_col0 + c0 : rhs_col0 + c0 + w],
                                         start=(k == 0), stop=last)
                    for g, (d, c0, w) in enumerate(xtr):
                        pt = tp.tile([P, NCH], f32, tag="tp")
                        nc.tensor.matmul(pt[:, :w], wt[k][:, ts(d, P)],
                                         rhsT[:, k, rhs_col0 + c0 : rhs_col0 + c0 + w],
                                         start=True, stop=True)
                        if k == 0:
                            nc.vector.tensor_copy(sacc[g][:, :w], pt[:, :w])
                        else:
                            nc.vector.tensor_add(sacc[g][:, :w], sacc[g][:, :w],
                                                 pt[:, :w])
                for ps, (d, c0, w) in zip(pss, blk):
                    finish_group(ps, d, c0, w)
                for g, (d, c0, w) in enumerate(xtr):
                    finish_group(sacc[g], d, c0, w)
                gi = stream_first + len(xtr)
            for d, c0, w in groups[gi:]:
                ps = acc.tile([P, NCH], f32, tag="acc")
                for k in range(KT):
                    last = (k == KT - 1) and (bias_sb is None)
                    nc.tensor.matmul(ps[:, :w], wt[k][:, ts(d, P)],
                                     rhsT[:, k, rhs_col0 + c0 : rhs_col0 + c0 + w],
                                     start=(k == 0), stop=last)
                finish_group(ps, d, c0, w)
            emit_pending(pending)
            pending = None
            nc.vector.tensor_mul(out_sb[:, out_tile0 : out_tile0 + ntiles, :],
                                 clsacc[:],
                                 coef_t[:, coef_off : coef_off + ntiles, :])

        # ---- len expert first (frees hsT for semT reuse) ----
        def dma_hsT_k(k):
            nc.sync.dma_start(hsT[:, k, :S], hsT_r[:, k, :])

        expert_window(wlen_d, hsT, 0, S, ST, blen_sb, clen_t, 0, C_syn,
                      dma_with=dma_hsT_k, stream_first=5, stream_extra=3)
        nc.gpsimd.dma_start(out_r[:, C_syn : C_syn + ST, :],
                            out_sb[:, C_syn : C_syn + ST, :])

        # ---- GCN layer 1: sup1 = hs @ Wg1 (k-outer: start on first tiles) ----
        w_g1 = load_wtiles(wg1_d)
        for n in range(NN):
            for mb in (range(0, 3), range(3, 6), range(6, 8)):
                pss = {}
                for m in mb:
                    psk = acc.tile([P, NCH], f32, tag="acc")
                    pss[m] = psk
                for k in range(KT):
                    for m in mb:
                        nc.tensor.matmul(pss[m][:], hsT[:, k, ts(m, P)],
                                         w_g1[k][:, ts(n, NCH)],
                                         start=(k == 0), stop=(k == KT - 1))
                for m in mb:
                    nc.any.tensor_copy(sup[:, m, ts(n, NCH)], pss[m][:])

        # ---- agg1 (transposed): x1T[d,tok] = relu(sup1^T @ A_hat^T) ----
        # lhsT = sup1 tile (contraction over source tokens on partitions),
        # rhs = adjT tile; the agg matmul itself produces x1T -> no transposes
        adjT = adjpool.tile([P, TT, S], bf16, tag="adjT")
        nc.sync.dma_start(adjT[:], adjT_r)
        # semT replaces hsT in the shared slot (loads during GCN phases)
        semT = hsempool.tile([P, KT, S + spad_sem], bf16, tag="hsem")
        for k in range(KT):
            if spad_sem:
                nc.vector.memset(semT[:, k, S:], 0.0)
        nc.sync.dma_start(semT[:, :, :S], semT_r)
        w_g2 = load_wtiles(wg2_d)
        for d in range(KT):
            for c in range(S // NCH):
                ps = acc.tile([P, NCH], f32, tag="acc")
                for t in range(TT):
                    nc.tensor.matmul(ps[:], sup[:, t, ts(d, P)],
                                     adjT[:, t, ts(c, NCH)],
                                     start=(t == 0), stop=(t == TT - 1))
                nc.scalar.activation(bigT[:, d, ts(c, NCH)], ps[:], AF.Relu)

        # ---- GCN layer 2: sup2 = x1 @ Wg2 ----
        for m in range(ST):
            for n in range(NN):
                ps = acc.tile([P, NCH], f32, tag="acc")
                for k in range(KT):
                    nc.tensor.matmul(ps[:], bigT[:, k, ts(m, P)],
                                     w_g2[k][:, ts(n, NCH)],
                                     start=(k == 0), stop=(k == KT - 1))
                nc.any.tensor_copy(sup[:, m, ts(n, NCH)], ps[:])

        # ---- agg2 + residual + LayerNorm -> sharedT (into bigT) ----
        # Transposes run one m behind so the LN chain (DVE/Act) overlaps the
        # next tile's agg matmuls instead of stalling PE.
        prev_xb = None
        for m in range(ST):
            hsm = rowf32.tile([P, H], f32, tag="hsm")
            nc.sync.dma_start(hsm[:], hs_r[:, m, :])
            x2row = transf.tile([P, H], f32, tag="x2row")
            for n in range(NN):
                ps = acc.tile([P, NCH], f32, tag="acc")
                for t in range(TT):
                    nc.tensor.matmul(ps[:], adjT[:, t, ts(m, P)],
                                     sup[:, t, ts(n, NCH)],
                                     start=(t == 0), stop=(t == TT - 1))
                nc.scalar.activation(x2row[:, ts(n, NCH)], ps[:], AF.Relu)
                if n == 0 and prev_xb is not None:
                    xb, pm = prev_xb
                    transpose_into(bigT, lambda k: xb[:, ts(k, P)], pm, id_bf)
            nc.vector.tensor_add(x2row[:], x2row[:], hsm[:])
            stats = small.tile([P, NN, 6], f32, tag="stats")
            for c in range(NN):
                nc.vector.bn_stats(stats[:, c, :], x2row[:, ts(c, NCH)])
            mv = small.tile([P, 2], f32, tag="mv")
            nc.vector.bn_aggr(mv[:], stats[:])
            rstd = small.tile([P, 1], f32, tag="rstd")
            nc.scalar.activation(rstd[:], mv[:, 1:2], AF.Sqrt, bias=eps_t[:])
            nc.vector.reciprocal(rstd[:], rstd[:])
            xb = transb.tile([P, H], bf16, tag="xrow")
            nc.vector.tensor_scalar(out=xb[:], in0=x2row[:],
                                    scalar1=mv[:, 0:1], scalar2=rstd[:],
                                    op0=ALU.subtract, op1=ALU.mult)
            prev_xb = (xb, m)
        xb, pm = prev_xb
        transpose_into(bigT, lambda k: xb[:, ts(k, P)], pm, id_bf)

        # ---- syn experts on sharedT (bigT) ----
        fo = 0
        for e in range(3):
            if caps_syn[e]:
                expert_window(wsyn_d[e], bigT, re_syn[e], spans_syn[e],
                              caps_syn[e], bsyn_sb[e], csyn_t, fo, fo)
                fo += caps_syn[e]
        if C_syn:
            nc.gpsimd.dma_start(out_r[:, 0:C_syn, :], out_sb[:, 0:C_syn, :])

        # ---- sem experts on semT ----
        fo = 0
        for e in range(3):
            if caps_sem[e]:
                expert_window(wsem_d[e], semT, re_sem[e], spans_sem[e],
                              caps_sem[e], bsem_sb[e], csem_t, fo,
                              C_syn + ST + fo)
                fo += caps_sem[e]
                nc.gpsimd.dma_start(
                    out_r[:, C_syn + ST + fo - caps_sem[e] : C_syn + ST + fo, :],
                    out_sb[:, C_syn + ST + fo - caps_sem[e] : C_syn + ST + fo, :])

    nc.compile()
    return nc


def _get_program(geom):
    if geom not in _prog_cache:
        _prog_cache[geom] = _build_program(geom)
    return _prog_cache[geom]


# ---------------------------------------------------------------- host glue
def _prepare(inputs):
    """Compute routing, permutations, windows; build per-core in_maps and
    decode metadata. Returns (geom, in_maps, meta)."""
    hs = np.asarray(inputs["hidden_states"], dtype=np.float32)
    adj = np.asarray(inputs["adj_matrix"], dtype=np.float32)
    seq_lengths = np.asarray(inputs["seq_lengths"])
    router_w = np.asarray(inputs["router_w"], dtype=np.float32)
    router_b = np.asarray(inputs["router_b"], dtype=np.float32)
    gcn1_w = np.asarray(inputs["gcn1_w"], dtype=np.float32)
    gcn2_w = np.asarray(inputs["gcn2_w"], dtype=np.float32)
    ln_g = np.asarray(inputs["ln_g"], dtype=np.float32)
    ln_b = np.asarray(inputs["ln_b"], dtype=np.float32)
    syn_w = np.asarray(inputs["syn_w"], dtype=np.float32)
    syn_b = np.asarray(inputs["syn_b"], dtype=np.float32)
    len_short_w = np.asarray(inputs["len_short_w"], dtype=np.float32)
    len_short_b = np.asarray(inputs["len_short_b"], dtype=np.float32)
    len_long_w = np.asarray(inputs["len_long_w"], dtype=np.float32)
    len_long_b = np.asarray(inputs["len_long_b"], dtype=np.float32)
    sem_w = np.asarray(inputs["sem_w"], dtype=np.float32)
    sem_b = np.asarray(inputs["sem_b"], dtype=np.float32)
    cls_w = np.asarray(inputs["cls_w"], dtype=np.float32)
    cls_b = np.asarray(inputs["cls_b"], dtype=np.float32)

    c_syn, syn_i, c_len, c_sem, sem_i, is_short = _route_host(
        hs, router_w, router_b, seq_lengths)

    # fold LN affine into syn weights: LN_plain(x) @ (g*W) + (b@W + bias)
    syn_w_f = (ln_g[None, :, None] * syn_w).astype(np.float32)
    syn_b_f = (syn_b + np.einsum("h,ehd->ed", ln_b, syn_w)).astype(np.float32)

    perm = np.argsort(syn_i, axis=1, kind="stable")          # [B,S]
    syn_i_p = np.take_along_axis(syn_i, perm, axis=1)
    sem_i_p = np.take_along_axis(sem_i, perm, axis=1)
    sem_perm = np.argsort(sem_i_p, axis=1, kind="stable")    # syn-order -> sem-order
    sem_i_s = np.take_along_axis(sem_i_p, sem_perm, axis=1)

    re_syn, caps_syn, spans_syn = _windows(syn_i_p)
    re_sem, caps_sem, spans_sem = _windows(sem_i_s)
    C_syn, C_sem = sum(caps_syn), sum(caps_sem)

    synb_nz = bool(np.any(syn_b_f != 0))
    lenb_nz = bool(np.any(len_short_b != 0) or np.any(len_long_b != 0))
    semb_nz = bool(np.any(sem_b != 0))
    geom = (re_syn, caps_syn, spans_syn, re_sem, caps_sem, spans_sem,
            synb_nz, lenb_nz, semb_nz)

    wg1 = np.ascontiguousarray(gcn1_w.astype(_BF16))
    wg2 = np.ascontiguousarray(gcn2_w.astype(_BF16))
    wsyn = np.ascontiguousarray(syn_w_f.astype(_BF16))
    wlen_s = np.ascontiguousarray(len_short_w.astype(_BF16))
    wlen_l = np.ascontiguousarray(len_long_w.astype(_BF16))
    wsem = np.ascontiguousarray(sem_w.astype(_BF16))
    wcls = np.ascontiguousarray(cls_w.astype(_BF16))

    def win_coef(cvec, idx_sorted_row, re, caps, grp):
        """Per-window masked coefficients, zero-padded to caps*P, duplicated
        over the 2 output classes so the device applies them in one mult."""
        out = np.zeros((max(sum(caps), 1) * P, 2), np.float32)
        off = 0
        for e in range(3):
            w = caps[e] * P
            lo = re[e]
            hi = min(S, lo + w)
            seg = np.where(idx_sorted_row[lo:hi] == e, cvec[lo:hi], 0.0)
            out[off : off + (hi - lo), 0] = seg
            out[off : off + (hi - lo), 1] = seg
            off += w
        return out

    in_maps = []
    meta = []
    for b in range(B):
        p = perm[b]
        sp = sem_perm[b]
        hs_p = hs[b][p]
        adj_p = adj[b][p][:, p]
        deg = np.maximum(adj_p.sum(axis=1, dtype=np.float32), 1e-9)
        adj_n = adj_p / deg[:, None]
        hs_sem = hs_p[sp]
        c_syn_p = c_syn[b][p]
        c_len_p = c_len[b][p]
        c_sem_s = c_sem[b][p][sp]

        m = {
            "hsT": np.ascontiguousarray(hs_p.T.astype(_BF16)),
            "adjT": np.ascontiguousarray(adj_n.T.astype(_BF16)),
            "hs": np.ascontiguousarray(hs_p),
            "semT": np.ascontiguousarray(hs_sem.T.astype(_BF16)),
            "wg1": wg1, "wg2": wg2, "wsyn": wsyn,
            "wlen": wlen_s if is_short[b] else wlen_l,
            "wsem": wsem, "wcls": wcls,
            "csyn": win_coef(c_syn_p, syn_i_p[b], re_syn, caps_syn, "syn"),
            "clen": np.ascontiguousarray(np.stack([c_len_p, c_len_p], axis=1)),
            "csem": win_coef(c_sem_s, sem_i_s[b], re_sem, caps_sem, "sem"),
        }
        if synb_nz:
            m["bsyn"] = syn_b_f
        if lenb_nz:
            m["blen"] = (len_short_b if is_short[b]
                         else len_long_b).reshape(1, H).astype(np.float32)
        if semb_nz:
            m["bsem"] = sem_b.astype(np.float32)
        in_maps.append(m)
        meta.append((p, sp))

    return geom, in_maps, meta, cls_b


def _decode(out_rows, geom, meta_b, cls_b):
    """out_rows: [NT*P, 2] device output for one core -> [S,2] original order."""
    re_syn, caps_syn, re_sem, caps_sem = geom[0], geom[1], geom[3], geom[4]
    C_syn, C_sem = sum(caps_syn), sum(caps_sem)
    p, sp = meta_b
    acc_syn = np.zeros((S, 2), np.float32)   # syn-order accumulation
    off = 0
    for e in range(3):
        w = caps_syn[e] * P
        lo = re_syn[e]
        hi = min(S, lo + w)
        acc_syn[lo:hi] += out_rows[off : off + (hi - lo)]
        off += w
    acc_syn += out_rows[C_syn * P : C_syn * P + S]          # len group
    acc_sem = np.zeros((S, 2), np.float32)   # sem-order
    off = (C_syn + ST) * P
    for e in range(3):
        w = caps_sem[e] * P
        lo = re_sem[e]
        hi = min(S, lo + w)
        acc_sem[lo:hi] += out_rows[off : off + (hi - lo)]
        off += w
    acc_syn[sp] += acc_sem
    res = np.empty((S, 2), np.float32)
    res[p] = acc_syn
    return res + cls_b


def kernel(**inputs):
    from concourse import bass_utils

    geom, in_maps, meta, cls_b = _prepare(inputs)
    nc = _get_program(geom)

    try:
        res = bass_utils.run_bass_kernel_spmd(nc, in_maps, core_ids=list(range(B)))
    except Exception:
        # transient device wedge (NRT_EXEC_UNIT_UNRECOVERABLE) clears on retry
        res = bass_utils.run_bass_kernel_spmd(nc, in_maps, core_ids=list(range(B)))
    globals()["_last_results"] = res
    out = np.stack([_decode(np.asarray(res.results[b]["out"], np.float32),
                            geom, meta[b], cls_b)
                    for b in range(B)]).astype(np.float32)
    return out


# revision 42
# speedup vs baseline: 1.0252x; 1.0003x over previous
"""Trainium2 Bass kernel for nn_MoEDetector (moe_routing).

Strategy: data-parallel over batch B=8 -> one batch per NeuronCore, plus
top-1 expert bucketing so the syn/sem groups run ~3/8 of the dense work.

Host side (cheap, exact):
  - router logits/probs/argmax + group coefficients in fp32 numpy
    (top-2 logit gaps are ~1e-4 while fp32 sum-order noise is ~1e-6, so
    the argmax always matches the jax reference)
  - tokens sorted by syn expert (perm applied to hs, adj rows+cols);
    second sort by sem expert gives hs_sem
  - per-expert compile-time column WINDOWS [re_e, re_e+cap_e*128) that
    cover the bucket on every core (offsets differ per core; the window
    union is compile-time, per-core masking via zeroed coefficients)
  - adjacency degree-normalized + transposed, hs transposed, both bf16
  - final per-group cls outputs are unpermuted and summed on host

Device program (shared by all 8 cores; per-core tensor CONTENT differs):
  - GCN1 -> agg1(relu) -> GCN2 -> agg2(relu) -> +hs residual -> LayerNorm
    (affine folded into syn weights), matmuls bf16, accumulation fp32
  - experts run TRANSPOSED: out_T[d,tok] = W^T @ x_T, so the gelu output
    lands pre-transposed for the cls projection and the per-token
    coefficient factors out of the d-contraction -> applied after cls as
    a per-partition scalar on the [slots,2] result
  - groups: syn (3 windows on sharedT), len (all tokens on hsT),
    sem (3 windows on hs_semT); each -> fusedT bf16 -> cls -> out rows
"""

import numpy as np
import ml_dtypes
from contextlib import ExitStack

B, S, H = 8, 1024, 1536
THRESHOLD = 128
NEG = -1e9
P = 128
ST = S // P          # 8 token tiles
KT = H // P          # 12 h tiles
TT = S // P          # 8 t tiles
NCH = 512            # matmul moving free-dim chunk
NN = H // NCH        # 3 chunks of H
EPS = 1e-5
SPAD = 384           # pad tail so expert windows may overrun S

_BF16 = ml_dtypes.bfloat16

_prog_cache = {}


# ---------------------------------------------------------------- host math
def _route_host(hs, rw, rb, seq_lengths):
    """fp32 numpy replication of the reference router."""
    logits = (hs.reshape(-1, H).astype(np.float32) @ rw).reshape(B, S, 8) + rb
    is_short = (np.asarray(seq_lengths) <= THRESHOLD)
    lg = logits.copy()
    lg[..., 3] = np.where(is_short[:, None], logits[..., 3], NEG)
    lg[..., 4] = np.where(is_short[:, None], NEG, logits[..., 4])
    m = lg.max(-1, keepdims=True)
    e = np.exp((lg - m).astype(np.float32))
    probs = (e / e.sum(-1, keepdims=True)).astype(np.float32)
    syn_p = probs[..., 0:3].max(-1)
    syn_i = probs[..., 0:3].argmax(-1)
    len_p = probs[..., 3:5].max(-1)
    sem_p = probs[..., 5:8].max(-1)
    sem_i = probs[..., 5:8].argmax(-1)
    den = syn_p + len_p + sem_p
    return ((syn_p / den).astype(np.float32), syn_i,
            (len_p / den).astype(np.float32),
            (sem_p / den).astype(np.float32), sem_i, is_short)


def _windows(idx_sorted):
    """idx_sorted: [B, S] expert index per token, sorted ascending per row.
    Returns (re, caps, spans): compile-time window starts, tile capacities
    (output layout), and exact column spans (matmul width) covering bucket
    e on every core."""
    re, caps, spans = [], [], []
    for e in range(3):
        starts = (idx_sorted < e).sum(axis=1)      # bucket start per core
        ends = (idx_sorted <= e).sum(axis=1)       # bucket end per core
        r = int(starts.min())
        hi = int(ends.max())
        span = max(0, hi - r)
        re.append(r)
        spans.append(span)
        caps.append(-(-span // P))                 # ceil
    return tuple(re), tuple(caps), tuple(spans)


# ---------------------------------------------------------------- device IR
def _build_program(geom):
    """geom = (re_syn, caps_syn, spans_syn, re_sem, caps_sem, spans_sem,
    synb_nz, lenb_nz, semb_nz)"""
    import concourse.bass as bass
    import concourse.tile as tile
    from concourse import bacc, masks, mybir

    (re_syn, caps_syn, spans_syn, re_sem, caps_sem, spans_sem,
     synb_nz, lenb_nz, semb_nz) = geom
    C_syn = sum(caps_syn)
    C_sem = sum(caps_sem)
    NT = C_syn + ST + C_sem                        # output tiles total
    spad_syn = max([0] + [re_syn[e] + spans_syn[e] - S for e in range(3)])
    spad_sem = max([0] + [re_sem[e] + spans_sem[e] - S for e in range(3)])
    f32 = mybir.dt.float32
    bf16 = mybir.dt.bfloat16
    AF = mybir.ActivationFunctionType
    ALU = mybir.AluOpType
    AX = mybir.AxisListType
    ts = bass.ts

    nc = bacc.Bacc("TRN2", target_bir_lowering=False, debug=False)

    # ---- DRAM I/O ----
    hsT_d = nc.dram_tensor("hsT", [H, S], bf16, kind="ExternalInput").ap()
    adjT_d = nc.dram_tensor("adjT", [S, S], bf16, kind="ExternalInput").ap()
    hs_d = nc.dram_tensor("hs", [S, H], f32, kind="ExternalInput").ap()
    semT_d = nc.dram_tensor("semT", [H, S], bf16, kind="ExternalInput").ap()
    wg1_d = nc.dram_tensor("wg1", [H, H], bf16, kind="ExternalInput").ap()
    wg2_d = nc.dram_tensor("wg2", [H, H], bf16, kind="ExternalInput").ap()
    wsyn_d = nc.dram_tensor("wsyn", [3, H, H], bf16, kind="ExternalInput").ap()
    wlen_d = nc.dram_tensor("wlen", [H, H], bf16, kind="ExternalInput").ap()
    wsem_d = nc.dram_tensor("wsem", [3, H, H], bf16, kind="ExternalInput").ap()
    wcls_d = nc.dram_tensor("wcls", [H, 2], bf16, kind="ExternalInput").ap()
    csyn_d = nc.dram_tensor("csyn", [max(C_syn, 1) * P, 2], f32, kind="ExternalInput").ap()
    clen_d = nc.dram_tensor("clen", [S, 2], f32, kind="ExternalInput").ap()
    csem_d = nc.dram_tensor("csem", [max(C_sem, 1) * P, 2], f32, kind="ExternalInput").ap()
    bsyn_d = nc.dram_tensor("bsyn", [3, H], f32, kind="ExternalInput").ap() if synb_nz else None
    blen_d = nc.dram_tensor("blen", [1, H], f32, kind="ExternalInput").ap() if lenb_nz else None
    bsem_d = nc.dram_tensor("bsem", [3, H], f32, kind="ExternalInput").ap() if semb_nz else None
    out_d = nc.dram_tensor("out", [NT * P, 2], f32, kind="ExternalOutput").ap()

    hsT_r = hsT_d.rearrange("(k p) s -> p k s", p=P)
    adjT_r = adjT_d.rearrange("(t p) s -> p t s", p=P)
    hs_r = hs_d.rearrange("(a p) h -> p a h", p=P)
    semT_r = semT_d.rearrange("(k p) s -> p k s", p=P)
    wcls_r = wcls_d.rearrange("(k p) c -> p k c", p=P)
    csyn_r = csyn_d.rearrange("(a p) c -> p a c", p=P)
    clen_r = clen_d.rearrange("(a p) c -> p a c", p=P)
    csem_r = csem_d.rearrange("(a p) c -> p a c", p=P)
    out_r = out_d.rearrange("(a p) c -> p a c", p=P)

    with tile.TileContext(nc) as tc, ExitStack() as ctx:
        # ---- pools ----
        const = ctx.enter_context(tc.tile_pool(name="const", bufs=1))
        hsempool = ctx.enter_context(tc.tile_pool(name="hsempool", bufs=1))
        adjpool = ctx.enter_context(tc.tile_pool(name="adjpool", bufs=1))
        suppool = ctx.enter_context(tc.tile_pool(name="suppool", bufs=1))
        bigpool = ctx.enter_context(tc.tile_pool(name="bigpool", bufs=1))
        wpool = ctx.enter_context(tc.tile_pool(name="wpool", bufs=24))
        small = ctx.enter_context(tc.tile_pool(name="small", bufs=2))
        transb = ctx.enter_context(tc.tile_pool(name="transb", bufs=2))
        transf = ctx.enter_context(tc.tile_pool(name="transf", bufs=2))
        rowf32 = ctx.enter_context(tc.tile_pool(name="rowf32", bufs=1))
        chunkp = ctx.enter_context(tc.tile_pool(name="chunkp", bufs=3))
        sbacc = ctx.enter_context(tc.tile_pool(name="sbacc", bufs=4))
        acc = ctx.enter_context(tc.tile_pool(name="acc", bufs=5, space="PSUM"))
        tp = ctx.enter_context(tc.tile_pool(name="tp", bufs=2, space="PSUM"))
        clsps = ctx.enter_context(tc.tile_pool(name="clsps", bufs=1, space="PSUM"))

        # ---- constants / small inputs ----
        id_bf = const.tile([P, P], bf16, tag="idb")
        masks.make_identity(nc, id_bf[:])
        eps_t = const.tile([P, 1], f32, tag="eps")
        nc.vector.memset(eps_t[:], EPS)
        wcls_sb = const.tile([P, KT, 2], bf16, tag="wcls")
        nc.gpsimd.dma_start(wcls_sb[:], wcls_r)
        csyn_t = const.tile([P, max(C_syn, 1), 2], f32, tag="csyn")
        nc.gpsimd.dma_start(csyn_t[:], csyn_r)
        clen_t = const.tile([P, ST, 2], f32, tag="clen")
        nc.gpsimd.dma_start(clen_t[:], clen_r)
        csem_t = const.tile([P, max(C_sem, 1), 2], f32, tag="csem")
        nc.gpsimd.dma_start(csem_t[:], csem_r)
        ones_row = None
        if synb_nz or lenb_nz or semb_nz:
            ones_row = const.tile([1, NCH], f32, tag="ones")
            nc.vector.memset(ones_row[:], 1.0)

        def bias_row(dram_ap, tag):
            t = const.tile([1, H], f32, tag=tag)
            nc.gpsimd.dma_start(t[:], dram_ap)
            return t

        bsyn_sb = ([bias_row(bsyn_d[e : e + 1, :], f"bsyn{e}") for e in range(3)]
                   if bsyn_d is not None else [None] * 3)
        blen_sb = bias_row(blen_d, "blen") if blen_d is not None else None
        bsem_sb = ([bias_row(bsem_d[e : e + 1, :], f"bsem{e}") for e in range(3)]
                   if bsem_d is not None else [None] * 3)

        # ---- big persistent SBUF tensors ----
        # hsT and (later) semT share one pool slot: hsT is dead after GCN1,
        # semT is only needed for the final sem expert group.
        hsT = hsempool.tile([P, KT, S + spad_sem], bf16, tag="hsem")
        bigT = bigpool.tile([P, KT, S + spad_syn], bf16, tag="bigT")
        for k in range(KT):
            if spad_syn:
                nc.vector.memset(bigT[:, k, S:], 0.0)
        sup = suppool.tile([P, ST, H], bf16, tag="sup")
        out_sb = small.tile([P, NT, 2], f32, tag="outsb")

        def load_wtiles(wdram):
            tiles = []
            for k in range(KT):
                wt = wpool.tile([P, H], bf16, tag="w")
                nc.sync.dma_start(wt[:], wdram[ts(k, P), :])
                tiles.append(wt)
            return tiles

        def transpose_into(dstT, src_of_k, m, ident):
            for k in range(KT):
                pt = tp.tile([P, P], ident.dtype, tag="tp")
                nc.tensor.transpose(pt[:], src_of_k(k), ident[:])
                nc.any.tensor_copy(dstT[:, k, ts(m, P)], pt[:])

        # ---- expert group helper (transposed form, fused cls) ----
        # One "window" = (rhsT, rhs_col0, ntiles, weight dram, bias, coef tile
        # column offset, out_sb tile offset). Emits, per (d, chunk):
        # expert matmuls -> [pipelined cls of the previous chunk] -> gelu.
        # cls partials accumulate over d into an SBUF accumulator; the
        # per-token coefficient is applied after cls (it factors out of the
        # d-contraction), so padding/masked slots simply get multiplied by 0.
        def expert_window(wdram, rhsT, rhs_col0, span, ntiles, bias_sb, coef_t,
                          coef_off, out_tile0, dma_with=None, stream_first=0,
                          stream_extra=0):
            wt = []
            for k in range(KT):
                if dma_with is not None:
                    dma_with(k)
                wk = wpool.tile([P, H], bf16, tag="w")
                nc.sync.dma_start(wk[:], wdram[ts(k, P), :])
                wt.append(wk)
            nch = [(c0, min(NCH, span - c0)) for c0 in range(0, span, NCH)]
            groups = [(d, c0, w) for d in range(KT) for (c0, w) in nch]
            clsacc = small.tile([P, ntiles, 2], f32, tag="clsacc")
            nc.vector.memset(clsacc[:], 0.0)
            pending = None

            def emit_pending(p):
                # full 128-wide tiles even when the span is partial: the tail
                # columns of ch hold stale-but-finite values (the len expert
                # fills the ring with 512-wide chunks first) and their cls
                # rows get multiplied by a zero coefficient at the end.
                ch, pc0, pw, pd = p
                nj = -(-pw // P)
                j0 = pc0 // P
                psd = clsps.tile([P, nj, 2], f32, tag="cls")
                for jj in range(nj):
                    nc.tensor.matmul(psd[:, jj, :], ch[:, ts(jj, P)],
                                     wcls_sb[:, pd, :],
                                     start=True, stop=True)
                nc.vector.tensor_add(clsacc[:, j0 : j0 + nj, :],
                                     clsacc[:, j0 : j0 + nj, :], psd[:])

            def finish_group(ps, d, c0, w):
                nonlocal pending
                if bias_sb is not None:
                    nc.tensor.matmul(ps[:, :w], bias_sb[:, ts(d, P)],
                                     ones_row[:, :w], start=False, stop=True)
                if pending is not None:
                    emit_pending(pending)
                ch = chunkp.tile([P, NCH], bf16, tag="ch")
                nc.scalar.activation(ch[:, :w], ps[:, :w], AF.Gelu)
                pending = (ch, c0, w, d)

            gi = 0
            if stream_first > 1:
                # k-outer over the first groups so matmuls overlap the initial
                # weight/activation DMA stream tile-by-tile. Groups beyond the
                # PSUM capacity accumulate per-k partials into SBUF via DVE
                # (single-shot matmuls through the otherwise-idle tp banks),
                # raising PE work per arriving tile to cover the DMA window.
                blk = groups[:stream_first]
                xtr = (groups[stream_first : stream_first + stream_extra]
                       if bias_sb is None else [])
                pss = []
                for _ in blk:
                    psk = acc.tile([P, NCH], f32, tag="acc")
                    pss.append(psk)
                sacc = []
                for _ in xtr:
                    sb_t = sbacc.tile([P, NCH], f32, tag="sb")
                    sacc.append(sb_t)
                for k in range(KT):
                    for ps, (d, c0, w) in zip(pss, blk):
                        last = (k == KT - 1) and (bias_sb is None)
                        nc.tensor.matmul(ps[:, :w], wt[k][:, ts(d, P)],
                                         rhsT[:, k, rhs_col0 + c0 : rhs_col0 + c0 + w],
                                         start=(k == 0), stop=last)
                    for g, (d, c0, w) in enumerate(xtr):
                        pt = tp.tile([P, NCH], f32, tag="tp")
                        nc.tensor.matmul(pt[:, :w], wt[k][:, ts(d, P)],
                                         rhsT[:, k, rhs_col0 + c0 : rhs_col0 + c0 + w],
                                         start=True, stop=True)
                        if k == 0:
                            nc.vector.tensor_copy(sacc[g][:, :w], pt[:, :w])
                        else:
                            nc.vector.tensor_add(sacc[g][:, :w], sacc[g][:, :w],
                                                 pt[:, :w])
                for ps, (d, c0, w) in zip(pss, blk):
                    finish_group(ps, d, c0, w)
                for g, (d, c0, w) in enumerate(xtr):
                    finish_group(sacc[g], d, c0, w)
                gi = stream_first + len(xtr)
            for d, c0, w in groups[gi:]:
                ps = acc.tile([P, NCH], f32, tag="acc")
                for k in range(KT):
                    last = (k == KT - 1) and (bias_sb is None)
                    nc.tensor.matmul(ps[:, :w], wt[k][:, ts(d, P)],
                                     rhsT[:, k, rhs_col0 + c0 : rhs_col0 + c0 + w],
                                     start=(k == 0), stop=last)
                finish_group(ps, d, c0, w)
            emit_pending(pending)
            pending = None
            nc.vector.tensor_mul(out_sb[:, out_tile0 : out_tile0 + ntiles, :],
                                 clsacc[:],
                                 coef_t[:, coef_off : coef_off + ntiles, :])

        # ---- len expert first (frees hsT for semT reuse) ----
        def dma_hsT_k(k):
            nc.sync.dma_start(hsT[:, k, :S], hsT_r[:, k, :])

        expert_window(wlen_d, hsT, 0, S, ST, blen_sb, clen_t, 0, C_syn,
                      dma_with=dma_hsT_k, stream_first=5, stream_extra=3)
        nc.gpsimd.dma_start(out_r[:, C_syn : C_syn + ST, :],
                            out_sb[:, C_syn : C_syn + ST, :])

        # ---- GCN layer 1: sup1 = hs @ Wg1 (k-outer: start on first tiles) ----
        w_g1 = load_wtiles(wg1_d)
        for n in range(NN):
            for mb in (range(0, 3), range(3, 6), range(6, 8)):
                pss = {}
                for m in mb:
                    psk = acc.tile([P, NCH], f32, tag="acc")
                    pss[m] = psk
                for k in range(KT):
                    for m in mb:
                        nc.tensor.matmul(pss[m][:], hsT[:, k, ts(m, P)],
                                         w_g1[k][:, ts(n, NCH)],
                                         start=(k == 0), stop=(k == KT - 1))
                for m in mb:
                    nc.any.tensor_copy(sup[:, m, ts(n, NCH)], pss[m][:])

        # ---- agg1 (transposed): x1T[d,tok] = relu(sup1^T @ A_hat^T) ----
        # lhsT = sup1 tile (contraction over source tokens on partitions),
        # rhs = adjT tile; the agg matmul itself produces x1T -> no transposes
        adjT = adjpool.tile([P, TT, S], bf16, tag="adjT")
        nc.sync.dma_start(adjT[:], adjT_r)
        # semT replaces hsT in the shared slot (loads during GCN phases)
        semT = hsempool.tile([P, KT, S + spad_sem], bf16, tag="hsem")
        for k in range(KT):
            if spad_sem:
                nc.vector.memset(semT[:, k, S:], 0.0)
        nc.sync.dma_start(semT[:, :, :S], semT_r)
        w_g2 = load_wtiles(wg2_d)
        for d in range(KT):
            for c in range(S // NCH):
                ps = acc.tile([P, NCH], f32, tag="acc")
                for t in range(TT):
                    nc.tensor.matmul(ps[:], sup[:, t, ts(d, P)],
                                     adjT[:, t, ts(c, NCH)],
                                     start=(t == 0), stop=(t == TT - 1))
                nc.scalar.activation(bigT[:, d, ts(c, NCH)], ps[:], AF.Relu)

        # ---- GCN layer 2: sup2 = x1 @ Wg2 ----
        for m in range(ST):
            for n in range(NN):
                ps = acc.tile([P, NCH], f32, tag="acc")
                for k in range(KT):
                    nc.tensor.matmul(ps[:], bigT[:, k, ts(m, P)],
                                     w_g2[k][:, ts(n, NCH)],
                                     start=(k == 0), stop=(k == KT - 1))
                nc.any.tensor_copy(sup[:, m, ts(n, NCH)], ps[:])

        # ---- agg2 + residual + LayerNorm -> sharedT (into bigT) ----
        # Transposes run one m behind so the LN chain (DVE/Act) overlaps the
        # next tile's agg matmuls instead of stalling PE.
        prev_xb = None
        for m in range(ST):
            hsm = rowf32.tile([P, H], f32, tag="hsm")
            nc.sync.dma_start(hsm[:], hs_r[:, m, :])
            x2row = transf.tile([P, H], f32, tag="x2row")
            for n in range(NN):
                ps = acc.tile([P, NCH], f32, tag="acc")
                for t in range(TT):
                    nc.tensor.matmul(ps[:], adjT[:, t, ts(m, P)],
                                     sup[:, t, ts(n, NCH)],
                                     start=(t == 0), stop=(t == TT - 1))
                nc.scalar.activation(x2row[:, ts(n, NCH)], ps[:], AF.Relu)
                if n == 0 and prev_xb is not None:
                    xb, pm = prev_xb
                    transpose_into(bigT, lambda k: xb[:, ts(k, P)], pm, id_bf)
            nc.vector.tensor_add(x2row[:], x2row[:], hsm[:])
            stats = small.tile([P, NN, 6], f32, tag="stats")
            for c in range(NN):
                nc.vector.bn_stats(stats[:, c, :], x2row[:, ts(c, NCH)])
            mv = small.tile([P, 2], f32, tag="mv")
            nc.vector.bn_aggr(mv[:], stats[:])
            rstd = small.tile([P, 1], f32, tag="rstd")
            nc.scalar.activation(rstd[:], mv[:, 1:2], AF.Sqrt, bias=eps_t[:])
            nc.vector.reciprocal(rstd[:], rstd[:])
            xb = transb.tile([P, H], bf16, tag="xrow")
            nc.vector.tensor_scalar(out=xb[:], in0=x2row[:],
                                    scalar1=mv[:, 0:1], scalar2=rstd[:],
                                    op0=ALU.subtract, op1=ALU.mult)
            prev_xb = (xb, m)
        xb, pm = prev_xb
        transpose_into(bigT, lambda k: xb[:, ts(k, P)], pm, id_bf)

        # ---- syn experts on sharedT (bigT) ----
        fo = 0
        for e in range(3):
            if caps_syn[e]:
                expert_window(wsyn_d[e], bigT, re_syn[e], spans_syn[e],
                              caps_syn[e], bsyn_sb[e], csyn_t, fo, fo)
                fo += caps_syn[e]
        if C_syn:
            nc.gpsimd.dma_start(out_r[:, 0:C_syn, :], out_sb[:, 0:C_syn, :])

        # ---- sem experts on semT ----
        fo = 0
        for e in range(3):
            if caps_sem[e]:
                expert_window(wsem_d[e], semT, re_sem[e], spans_sem[e],
                              caps_sem[e], bsem_sb[e], csem_t, fo,
                              C_syn + ST + fo)
                fo += caps_sem[e]
                nc.gpsimd.dma_start(
                    out_r[:, C_syn + ST + fo - caps_sem[e] : C_syn + ST + fo, :],
                    out_sb[:, C_syn + ST + fo - caps_sem[e] : C_syn + ST + fo, :])

    nc.compile()
    return nc


def _get_program(geom):
    if geom not in _prog_cache:
        _prog_cache[geom] = _build_program(geom)
    return _prog_cache[geom]


# ---------------------------------------------------------------- host glue
def _prepare(inputs):
    """Compute routing, permutations, windows; build per-core in_maps and
    decode metadata. Returns (geom, in_maps, meta)."""
    hs = np.asarray(inputs["hidden_states"], dtype=np.float32)
    adj = np.asarray(inputs["adj_matrix"], dtype=np.float32)
    seq_lengths = np.asarray(inputs["seq_lengths"])
    router_w = np.asarray(inputs["router_w"], dtype=np.float32)
    router_b = np.asarray(inputs["router_b"], dtype=np.float32)
    gcn1_w = np.asarray(inputs["gcn1_w"], dtype=np.float32)
    gcn2_w = np.asarray(inputs["gcn2_w"], dtype=np.float32)
    ln_g = np.asarray(inputs["ln_g"], dtype=np.float32)
    ln_b = np.asarray(inputs["ln_b"], dtype=np.float32)
    syn_w = np.asarray(inputs["syn_w"], dtype=np.float32)
    syn_b = np.asarray(inputs["syn_b"], dtype=np.float32)
    len_short_w = np.asarray(inputs["len_short_w"], dtype=np.float32)
    len_short_b = np.asarray(inputs["len_short_b"], dtype=np.float32)
    len_long_w = np.asarray(inputs["len_long_w"], dtype=np.float32)
    len_long_b = np.asarray(inputs["len_long_b"], dtype=np.float32)
    sem_w = np.asarray(inputs["sem_w"], dtype=np.float32)
    sem_b = np.asarray(inputs["sem_b"], dtype=np.float32)
    cls_w = np.asarray(inputs["cls_w"], dtype=np.float32)
    cls_b = np.asarray(inputs["cls_b"], dtype=np.float32)

    c_syn, syn_i, c_len, c_sem, sem_i, is_short = _route_host(
        hs, router_w, router_b, seq_lengths)

    # fold LN affine into syn weights: LN_plain(x) @ (g*W) + (b@W + bias)
    syn_w_f = (ln_g[None, :, None] * syn_w).astype(np.float32)
    syn_b_f = (syn_b + np.einsum("h,ehd->ed", ln_b, syn_w)).astype(np.float32)

    perm = np.argsort(syn_i, axis=1, kind="stable")          # [B,S]
    syn_i_p = np.take_along_axis(syn_i, perm, axis=1)
    sem_i_p = np.take_along_axis(sem_i, perm, axis=1)
    sem_perm = np.argsort(sem_i_p, axis=1, kind="stable")    # syn-order -> sem-order
    sem_i_s = np.take_along_axis(sem_i_p, sem_perm, axis=1)

    re_syn, caps_syn, spans_syn = _windows(syn_i_p)
    re_sem, caps_sem, spans_sem = _windows(sem_i_s)
    C_syn, C_sem = sum(caps_syn), sum(caps_sem)

    synb_nz = bool(np.any(syn_b_f != 0))
    lenb_nz = bool(np.any(len_short_b != 0) or np.any(len_long_b != 0))
    semb_nz = bool(np.any(sem_b != 0))
    geom = (re_syn, caps_syn, spans_syn, re_sem, caps_sem, spans_sem,
            synb_nz, lenb_nz, semb_nz)

    wg1 = np.ascontiguousarray(gcn1_w.astype(_BF16))
    wg2 = np.ascontiguousarray(gcn2_w.astype(_BF16))
    wsyn = np.ascontiguousarray(syn_w_f.astype(_BF16))
    wlen_s = np.ascontiguousarray(len_short_w.astype(_BF16))
    wlen_l = np.ascontiguousarray(len_long_w.astype(_BF16))
    wsem = np.ascontiguousarray(sem_w.astype(_BF16))
    wcls = np.ascontiguousarray(cls_w.astype(_BF16))

    def win_coef(cvec, idx_sorted_row, re, caps, grp):
        """Per-window masked coefficients, zero-padded to caps*P, duplicated
        over the 2 output classes so the device applies them in one mult."""
        out = np.zeros((max(sum(caps), 1) * P, 2), np.float32)
        off = 0
        for e in range(3):
            w = caps[e] * P
            lo = re[e]
            hi = min(S, lo + w)
            seg = np.where(idx_sorted_row[lo:hi] == e, cvec[lo:hi], 0.0)
            out[off : off + (hi - lo), 0] = seg
            out[off : off + (hi - lo), 1] = seg
            off += w
        return out

    in_maps = []
    meta = []
    for b in range(B):
        p = perm[b]
        sp = sem_perm[b]
        hs_p = hs[b][p]
        adj_p = adj[b][p][:, p]
        deg = np.maximum(adj_p.sum(axis=1, dtype=np.float32), 1e-9)
        adj_n = adj_p / deg[:, None]
        hs_sem = hs_p[sp]
        c_syn_p = c_syn[b][p]
        c_len_p = c_len[b][p]
        c_sem_s = c_sem[b][p][sp]

        m = {
            "hsT": np.ascontiguousarray(hs_p.T.astype(_BF16)),
            "adjT": np.ascontiguousarray(adj_n.T.astype(_BF16)),
            "hs": np.ascontiguousarray(hs_p),
            "semT": np.ascontiguousarray(hs_sem.T.astype(_BF16)),
            "wg1": wg1, "wg2": wg2, "wsyn": wsyn,
            "wlen": wlen_s if is_short[b] else wlen_l,
            "wsem": wsem, "wcls": wcls,
            "csyn": win_coef(c_syn_p, syn_i_p[b], re_syn, caps_syn, "syn"),
            "clen": np.ascontiguousarray(np.stack([c_len_p, c_len_p], axis=1)),
            "csem": win_coef(c_sem_s, sem_i_s[b], re_sem, caps_sem, "sem"),
        }
        if synb_nz:
            m["bsyn"] = syn_b_f
        if lenb_nz:
            m["blen"] = (len_short_b if is_short[b]
                         else len_long_b).reshape(1, H).astype(np.float32)
        if semb_nz:
            m["bsem"] = sem_b.astype(np.float32)
        in_maps.append(m)
        meta.append((p, sp))

    return geom, in_maps, meta, cls_b


def _decode(out_rows, geom, meta_b, cls_b):
    """out_rows: [NT*P, 2] device output for one core -> [S,2] original order."""
    re_syn, caps_syn, re_sem, caps_sem = geom[0], geom[1], geom[3], geom[4]
    C_syn, C_sem = sum(caps_syn), sum(caps_sem)
    p, sp = meta_b
    acc_syn = np.zeros((S, 2), np.float32)   # syn-order accumulation
    off = 0
    for e in range(3):
        w = caps_syn[e] * P
        lo = re_syn[e]
        hi = min(S, lo + w)
        acc_syn[lo:hi] += out_rows[off : off + (hi - lo)]
        off += w
    acc_syn += out_rows[C_syn * P : C_syn * P + S]          # len group
    acc_sem = np.zeros((S, 2), np.float32)   # sem-order
    off = (C_syn + ST) * P
    for e in range(3):
        w = caps_sem[e] * P
        lo = re_sem[e]
        hi = min(S, lo + w)
        acc_sem[lo:hi] += out_rows[off : off + (hi - lo)]
        off += w
    acc_syn[sp] += acc_sem
    res = np.empty((S, 2), np.float32)
    res[p] = acc_syn
    return res + cls_b


def kernel(**inputs):
    from concourse import bass_utils

    geom, in_maps, meta, cls_b = _prepare(inputs)
    nc = _get_program(geom)

    try:
        res = bass_utils.run_bass_kernel_spmd(nc, in_maps, core_ids=list(range(B)))
    except Exception:
        # transient device wedge (NRT_EXEC_UNIT_UNRECOVERABLE) clears on retry
        res = bass_utils.run_bass_kernel_spmd(nc, in_maps, core_ids=list(range(B)))
    globals()["_last_results"] = res
    out = np.stack([_decode(np.asarray(res.results[b]["out"], np.float32),
                            geom, meta[b], cls_b)
                    for b in range(B)]).astype(np.float32)
    return out


# revision 49
# speedup vs baseline: 1.0310x; 1.0056x over previous
"""Trainium2 Bass kernel for nn_MoEDetector (moe_routing).

Strategy: data-parallel over batch B=8 -> one batch per NeuronCore, plus
top-1 expert bucketing so the syn/sem groups run ~3/8 of the dense work.

Host side (cheap, exact):
  - router logits/probs/argmax + group coefficients in fp32 numpy
    (top-2 logit gaps are ~1e-4 while fp32 sum-order noise is ~1e-6, so
    the argmax always matches the jax reference)
  - tokens sorted by syn expert (perm applied to hs, adj rows+cols);
    second sort by sem expert gives hs_sem
  - per-expert compile-time column WINDOWS [re_e, re_e+cap_e*128) that
    cover the bucket on every core (offsets differ per core; the window
    union is compile-time, per-core masking via zeroed coefficients)
  - adjacency degree-normalized + transposed, hs transposed, both bf16
  - final per-group cls outputs are unpermuted and summed on host

Device program (shared by all 8 cores; per-core tensor CONTENT differs):
  - GCN1 -> agg1(relu) -> GCN2 -> agg2(relu) -> +hs residual -> LayerNorm
    (affine folded into syn weights), matmuls bf16, accumulation fp32
  - experts run TRANSPOSED: out_T[d,tok] = W^T @ x_T, so the gelu output
    lands pre-transposed for the cls projection and the per-token
    coefficient factors out of the d-contraction -> applied after cls as
    a per-partition scalar on the [slots,2] result
  - groups: syn (3 windows on sharedT), len (all tokens on hsT),
    sem (3 windows on hs_semT); each -> fusedT bf16 -> cls -> out rows
"""

import numpy as np
import ml_dtypes
from contextlib import ExitStack

B, S, H = 8, 1024, 1536
THRESHOLD = 128
NEG = -1e9
P = 128
ST = S // P          # 8 token tiles
KT = H // P          # 12 h tiles
TT = S // P          # 8 t tiles
NCH = 512            # matmul moving free-dim chunk
NN = H // NCH        # 3 chunks of H
EPS = 1e-5
SPAD = 384           # pad tail so expert windows may overrun S

_BF16 = ml_dtypes.bfloat16

_prog_cache = {}


# ---------------------------------------------------------------- host math
def _route_host(hs, rw, rb, seq_lengths):
    """fp32 numpy replication of the reference router."""
    logits = (hs.reshape(-1, H).astype(np.float32) @ rw).reshape(B, S, 8) + rb
    is_short = (np.asarray(seq_lengths) <= THRESHOLD)
    lg = logits.copy()
    lg[..., 3] = np.where(is_short[:, None], logits[..., 3], NEG)
    lg[..., 4] = np.where(is_short[:, None], NEG, logits[..., 4])
    m = lg.max(-1, keepdims=True)
    e = np.exp((lg - m).astype(np.float32))
    probs = (e / e.sum(-1, keepdims=True)).astype(np.float32)
    syn_p = probs[..., 0:3].max(-1)
    syn_i = probs[..., 0:3].argmax(-1)
    len_p = probs[..., 3:5].max(-1)
    sem_p = probs[..., 5:8].max(-1)
    sem_i = probs[..., 5:8].argmax(-1)
    den = syn_p + len_p + sem_p
    return ((syn_p / den).astype(np.float32), syn_i,
            (len_p / den).astype(np.float32),
            (sem_p / den).astype(np.float32), sem_i, is_short)


def _windows(idx_sorted):
    """idx_sorted: [B, S] expert index per token, sorted ascending per row.
    Returns (re, caps, spans): compile-time window starts, tile capacities
    (output layout), and exact column spans (matmul width) covering bucket
    e on every core."""
    re, caps, spans = [], [], []
    for e in range(3):
        starts = (idx_sorted < e).sum(axis=1)      # bucket start per core
        ends = (idx_sorted <= e).sum(axis=1)       # bucket end per core
        r = int(starts.min())
        hi = int(ends.max())
        span = max(0, hi - r)
        re.append(r)
        spans.append(span)
        caps.append(-(-span // P))                 # ceil
    return tuple(re), tuple(caps), tuple(spans)


# ---------------------------------------------------------------- device IR
def _build_program(geom):
    """geom = (re_syn, caps_syn, spans_syn, re_sem, caps_sem, spans_sem,
    synb_nz, lenb_nz, semb_nz)"""
    import concourse.bass as bass
    import concourse.tile as tile
    from concourse import bacc, masks, mybir

    (re_syn, caps_syn, spans_syn, re_sem, caps_sem, spans_sem,
     synb_nz, lenb_nz, semb_nz) = geom
    C_syn = sum(caps_syn)
    C_sem = sum(caps_sem)
    NT = C_syn + ST + C_sem                        # output tiles total
    spad_syn = max([0] + [re_syn[e] + spans_syn[e] - S for e in range(3)])
    spad_sem = max([0] + [re_sem[e] + spans_sem[e] - S for e in range(3)])
    f32 = mybir.dt.float32
    bf16 = mybir.dt.bfloat16
    AF = mybir.ActivationFunctionType
    ALU = mybir.AluOpType
    AX = mybir.AxisListType
    ts = bass.ts

    nc = bacc.Bacc("TRN2", target_bir_lowering=False, debug=False)

    # ---- DRAM I/O ----
    hsT_d = nc.dram_tensor("hsT", [H, S], bf16, kind="ExternalInput").ap()
    adjT_d = nc.dram_tensor("adjT", [S, S], bf16, kind="ExternalInput").ap()
    hs_d = nc.dram_tensor("hs", [S, H], f32, kind="ExternalInput").ap()
    semT_d = nc.dram_tensor("semT", [H, S], bf16, kind="ExternalInput").ap()
    wg1_d = nc.dram_tensor("wg1", [H, H], bf16, kind="ExternalInput").ap()
    wg2_d = nc.dram_tensor("wg2", [H, H], bf16, kind="ExternalInput").ap()
    wsyn_d = nc.dram_tensor("wsyn", [3, H, H], bf16, kind="ExternalInput").ap()
    wlen_d = nc.dram_tensor("wlen", [H, H], bf16, kind="ExternalInput").ap()
    wsem_d = nc.dram_tensor("wsem", [3, H, H], bf16, kind="ExternalInput").ap()
    wcls_d = nc.dram_tensor("wcls", [H, 2], bf16, kind="ExternalInput").ap()
    csyn_d = nc.dram_tensor("csyn", [max(C_syn, 1) * P, 2], f32, kind="ExternalInput").ap()
    clen_d = nc.dram_tensor("clen", [S, 2], f32, kind="ExternalInput").ap()
    csem_d = nc.dram_tensor("csem", [max(C_sem, 1) * P, 2], f32, kind="ExternalInput").ap()
    bsyn_d = nc.dram_tensor("bsyn", [3, H], f32, kind="ExternalInput").ap() if synb_nz else None
    blen_d = nc.dram_tensor("blen", [1, H], f32, kind="ExternalInput").ap() if lenb_nz else None
    bsem_d = nc.dram_tensor("bsem", [3, H], f32, kind="ExternalInput").ap() if semb_nz else None
    out_d = nc.dram_tensor("out", [NT * P, 2], f32, kind="ExternalOutput").ap()

    hsT_r = hsT_d.rearrange("(k p) s -> p k s", p=P)
    adjT_r = adjT_d.rearrange("(t p) s -> p t s", p=P)
    hs_r = hs_d.rearrange("(a p) h -> p a h", p=P)
    semT_r = semT_d.rearrange("(k p) s -> p k s", p=P)
    wcls_r = wcls_d.rearrange("(k p) c -> p k c", p=P)
    csyn_r = csyn_d.rearrange("(a p) c -> p a c", p=P)
    clen_r = clen_d.rearrange("(a p) c -> p a c", p=P)
    csem_r = csem_d.rearrange("(a p) c -> p a c", p=P)
    out_r = out_d.rearrange("(a p) c -> p a c", p=P)

    with tile.TileContext(nc) as tc, ExitStack() as ctx:
        # ---- pools ----
        const = ctx.enter_context(tc.tile_pool(name="const", bufs=1))
        hsempool = ctx.enter_context(tc.tile_pool(name="hsempool", bufs=1))
        adjpool = ctx.enter_context(tc.tile_pool(name="adjpool", bufs=1))
        suppool = ctx.enter_context(tc.tile_pool(name="suppool", bufs=1))
        bigpool = ctx.enter_context(tc.tile_pool(name="bigpool", bufs=1))
        wpool = ctx.enter_context(tc.tile_pool(name="wpool", bufs=24))
        small = ctx.enter_context(tc.tile_pool(name="small", bufs=2))
        transb = ctx.enter_context(tc.tile_pool(name="transb", bufs=2))
        transf = ctx.enter_context(tc.tile_pool(name="transf", bufs=2))
        rowf32 = ctx.enter_context(tc.tile_pool(name="rowf32", bufs=1))
        chunkp = ctx.enter_context(tc.tile_pool(name="chunkp", bufs=3))
        sbacc = ctx.enter_context(tc.tile_pool(name="sbacc", bufs=4))
        acc = ctx.enter_context(tc.tile_pool(name="acc", bufs=5, space="PSUM"))
        tp = ctx.enter_context(tc.tile_pool(name="tp", bufs=2, space="PSUM"))
        clsps = ctx.enter_context(tc.tile_pool(name="clsps", bufs=1, space="PSUM"))

        # ---- constants / small inputs ----
        id_bf = const.tile([P, P], bf16, tag="idb")
        masks.make_identity(nc, id_bf[:])
        # PE p-state warmup: ~3us of no-DMA transposes during the initial
        # weight/activation DMA wait, so the stream matmuls start at full
        # clock instead of ramping inside the DMA-limited window
        for _ in range(16):
            ptw = tp.tile([P, P], bf16, tag="tp")
            nc.tensor.transpose(ptw[:], id_bf[:], id_bf[:])
        eps_t = const.tile([P, 1], f32, tag="eps")
        nc.vector.memset(eps_t[:], EPS)
        wcls_sb = const.tile([P, KT, 2], bf16, tag="wcls")
        nc.gpsimd.dma_start(wcls_sb[:], wcls_r)
        csyn_t = const.tile([P, max(C_syn, 1), 2], f32, tag="csyn")
        nc.gpsimd.dma_start(csyn_t[:], csyn_r)
        clen_t = const.tile([P, ST, 2], f32, tag="clen")
        nc.gpsimd.dma_start(clen_t[:], clen_r)
        csem_t = const.tile([P, max(C_sem, 1), 2], f32, tag="csem")
        nc.gpsimd.dma_start(csem_t[:], csem_r)
        ones_row = None
        if synb_nz or lenb_nz or semb_nz:
            ones_row = const.tile([1, NCH], f32, tag="ones")
            nc.vector.memset(ones_row[:], 1.0)

        def bias_row(dram_ap, tag):
            t = const.tile([1, H], f32, tag=tag)
            nc.gpsimd.dma_start(t[:], dram_ap)
            return t

        bsyn_sb = ([bias_row(bsyn_d[e : e + 1, :], f"bsyn{e}") for e in range(3)]
                   if bsyn_d is not None else [None] * 3)
        blen_sb = bias_row(blen_d, "blen") if blen_d is not None else None
        bsem_sb = ([bias_row(bsem_d[e : e + 1, :], f"bsem{e}") for e in range(3)]
                   if bsem_d is not None else [None] * 3)

        # ---- big persistent SBUF tensors ----
        # hsT and (later) semT share one pool slot: hsT is dead after GCN1,
        # semT is only needed for the final sem expert group.
        hsT = hsempool.tile([P, KT, S + spad_sem], bf16, tag="hsem")
        bigT = bigpool.tile([P, KT, S + spad_syn], bf16, tag="bigT")
        for k in range(KT):
            if spad_syn:
                nc.vector.memset(bigT[:, k, S:], 0.0)
        sup = suppool.tile([P, ST, H], bf16, tag="sup")
        out_sb = small.tile([P, NT, 2], f32, tag="outsb")

        def load_wtiles(wdram):
            tiles = []
            for k in range(KT):
                wt = wpool.tile([P, H], bf16, tag="w")
                nc.sync.dma_start(wt[:], wdram[ts(k, P), :])
                tiles.append(wt)
            return tiles

        def transpose_into(dstT, src_of_k, m, ident):
            for k in range(KT):
                pt = tp.tile([P, P], ident.dtype, tag="tp")
                nc.tensor.transpose(pt[:], src_of_k(k), ident[:])
                nc.any.tensor_copy(dstT[:, k, ts(m, P)], pt[:])

        # ---- expert group helper (transposed form, fused cls) ----
        # One "window" = (rhsT, rhs_col0, ntiles, weight dram, bias, coef tile
        # column offset, out_sb tile offset). Emits, per (d, chunk):
        # expert matmuls -> [pipelined cls of the previous chunk] -> gelu.
        # cls partials accumulate over d into an SBUF accumulator; the
        # per-token coefficient is applied after cls (it factors out of the
        # d-contraction), so padding/masked slots simply get multiplied by 0.
        def expert_window(wdram, rhsT, rhs_col0, span, ntiles, bias_sb, coef_t,
                          coef_off, out_tile0, dma_with=None, stream_first=0,
                          stream_extra=0):
            wt = []
            for k in range(KT):
                if dma_with is not None:
                    dma_with(k)
                wk = wpool.tile([P, H], bf16, tag="w")
                nc.sync.dma_start(wk[:], wdram[ts(k, P), :])
                wt.append(wk)
            nch = [(c0, min(NCH, span - c0)) for c0 in range(0, span, NCH)]
            groups = [(d, c0, w) for d in range(KT) for (c0, w) in nch]
            clsacc = small.tile([P, ntiles, 2], f32, tag="clsacc")
            nc.vector.memset(clsacc[:], 0.0)
            pending = None

            def emit_pending(p):
                # full 128-wide tiles even when the span is partial: the tail
                # columns of ch hold stale-but-finite values (the len expert
                # fills the ring with 512-wide chunks first) and their cls
                # rows get multiplied by a zero coefficient at the end.
                ch, pc0, pw, pd = p
                nj = -(-pw // P)
                j0 = pc0 // P
                psd = clsps.tile([P, nj, 2], f32, tag="cls")
                for jj in range(nj):
                    nc.tensor.matmul(psd[:, jj, :], ch[:, ts(jj, P)],
                                     wcls_sb[:, pd, :],
                                     start=True, stop=True)
                nc.vector.tensor_add(clsacc[:, j0 : j0 + nj, :],
                                     clsacc[:, j0 : j0 + nj, :], psd[:])

            def finish_group(ps, d, c0, w):
                nonlocal pending
                if bias_sb is not None:
                    nc.tensor.matmul(ps[:, :w], bias_sb[:, ts(d, P)],
                                     ones_row[:, :w], start=False, stop=True)
                if pending is not None:
                    emit_pending(pending)
                ch = chunkp.tile([P, NCH], bf16, tag="ch")
                nc.scalar.activation(ch[:, :w], ps[:, :w], AF.Gelu)
                pending = (ch, c0, w, d)

            gi = 0
            if stream_first > 1:
                # k-outer over the first groups so matmuls overlap the initial
                # weight/activation DMA stream tile-by-tile. Groups beyond the
                # PSUM capacity accumulate per-k partials into SBUF via DVE
                # (single-shot matmuls through the otherwise-idle tp banks),
                # raising PE work per arriving tile to cover the DMA window.
                blk = groups[:stream_first]
                xtr = (groups[stream_first : stream_first + stream_extra]
                       if bias_sb is None else [])
                pss = []
                for _ in blk:
                    psk = acc.tile([P, NCH], f32, tag="acc")
                    pss.append(psk)
                sacc = []
                for _ in xtr:
                    sb_t = sbacc.tile([P, NCH], f32, tag="sb")
                    sacc.append(sb_t)
                for k in range(KT):
                    for ps, (d, c0, w) in zip(pss, blk):
                        last = (k == KT - 1) and (bias_sb is None)
                        nc.tensor.matmul(ps[:, :w], wt[k][:, ts(d, P)],
                                         rhsT[:, k, rhs_col0 + c0 : rhs_col0 + c0 + w],
                                         start=(k == 0), stop=last)
                    for g, (d, c0, w) in enumerate(xtr):
                        pt = tp.tile([P, NCH], f32, tag="tp")
                        nc.tensor.matmul(pt[:, :w], wt[k][:, ts(d, P)],
                                         rhsT[:, k, rhs_col0 + c0 : rhs_col0 + c0 + w],
                                         start=True, stop=True)
                        if k == 0:
                            nc.vector.tensor_copy(sacc[g][:, :w], pt[:, :w])
                        else:
                            nc.vector.tensor_add(sacc[g][:, :w], sacc[g][:, :w],
                                                 pt[:, :w])
                for ps, (d, c0, w) in zip(pss, blk):
                    finish_group(ps, d, c0, w)
                for g, (d, c0, w) in enumerate(xtr):
                    finish_group(sacc[g], d, c0, w)
                gi = stream_first + len(xtr)
            for d, c0, w in groups[gi:]:
                ps = acc.tile([P, NCH], f32, tag="acc")
                for k in range(KT):
                    last = (k == KT - 1) and (bias_sb is None)
                    nc.tensor.matmul(ps[:, :w], wt[k][:, ts(d, P)],
                                     rhsT[:, k, rhs_col0 + c0 : rhs_col0 + c0 + w],
                                     start=(k == 0), stop=last)
                finish_group(ps, d, c0, w)
            emit_pending(pending)
            pending = None
            nc.vector.tensor_mul(out_sb[:, out_tile0 : out_tile0 + ntiles, :],
                                 clsacc[:],
                                 coef_t[:, coef_off : coef_off + ntiles, :])

        # ---- len expert first (frees hsT for semT reuse) ----
        def dma_hsT_k(k):
            nc.sync.dma_start(hsT[:, k, :S], hsT_r[:, k, :])

        expert_window(wlen_d, hsT, 0, S, ST, blen_sb, clen_t, 0, C_syn,
                      dma_with=dma_hsT_k, stream_first=5, stream_extra=3)
        nc.gpsimd.dma_start(out_r[:, C_syn : C_syn + ST, :],
                            out_sb[:, C_syn : C_syn + ST, :])

        # ---- GCN layer 1: sup1 = hs @ Wg1 (k-outer: start on first tiles) ----
        w_g1 = load_wtiles(wg1_d)
        for n in range(NN):
            for mb in (range(0, 3), range(3, 6), range(6, 8)):
                pss = {}
                for m in mb:
                    psk = acc.tile([P, NCH], f32, tag="acc")
                    pss[m] = psk
                for k in range(KT):
                    for m in mb:
                        nc.tensor.matmul(pss[m][:], hsT[:, k, ts(m, P)],
                                         w_g1[k][:, ts(n, NCH)],
                                         start=(k == 0), stop=(k == KT - 1))
                for m in mb:
                    nc.any.tensor_copy(sup[:, m, ts(n, NCH)], pss[m][:])

        # ---- agg1 (transposed): x1T[d,tok] = relu(sup1^T @ A_hat^T) ----
        # lhsT = sup1 tile (contraction over source tokens on partitions),
        # rhs = adjT tile; the agg matmul itself produces x1T -> no transposes
        adjT = adjpool.tile([P, TT, S], bf16, tag="adjT")
        nc.sync.dma_start(adjT[:], adjT_r)
        # semT replaces hsT in the shared slot (loads during GCN phases)
        semT = hsempool.tile([P, KT, S + spad_sem], bf16, tag="hsem")
        for k in range(KT):
            if spad_sem:
                nc.vector.memset(semT[:, k, S:], 0.0)
        nc.sync.dma_start(semT[:, :, :S], semT_r)
        w_g2 = load_wtiles(wg2_d)
        for d in range(KT):
            for c in range(S // NCH):
                ps = acc.tile([P, NCH], f32, tag="acc")
                for t in range(TT):
                    nc.tensor.matmul(ps[:], sup[:, t, ts(d, P)],
                                     adjT[:, t, ts(c, NCH)],
                                     start=(t == 0), stop=(t == TT - 1))
                nc.scalar.activation(bigT[:, d, ts(c, NCH)], ps[:], AF.Relu)

        # ---- GCN layer 2: sup2 = x1 @ Wg2 ----
        for m in range(ST):
            for n in range(NN):
                ps = acc.tile([P, NCH], f32, tag="acc")
                for k in range(KT):
                    nc.tensor.matmul(ps[:], bigT[:, k, ts(m, P)],
                                     w_g2[k][:, ts(n, NCH)],
                                     start=(k == 0), stop=(k == KT - 1))
                nc.any.tensor_copy(sup[:, m, ts(n, NCH)], ps[:])

        # ---- agg2 + residual + LayerNorm -> sharedT (into bigT) ----
        # Transposes run one m behind so the LN chain (DVE/Act) overlaps the
        # next tile's agg matmuls instead of stalling PE.
        prev_xb = None
        for m in range(ST):
            hsm = rowf32.tile([P, H], f32, tag="hsm")
            nc.sync.dma_start(hsm[:], hs_r[:, m, :])
            x2row = transf.tile([P, H], f32, tag="x2row")
            for n in range(NN):
                ps = acc.tile([P, NCH], f32, tag="acc")
                for t in range(TT):
                    nc.tensor.matmul(ps[:], adjT[:, t, ts(m, P)],
                                     sup[:, t, ts(n, NCH)],
                                     start=(t == 0), stop=(t == TT - 1))
                nc.scalar.activation(x2row[:, ts(n, NCH)], ps[:], AF.Relu)
                if n == 0 and prev_xb is not None:
                    xb, pm = prev_xb
                    transpose_into(bigT, lambda k: xb[:, ts(k, P)], pm, id_bf)
            nc.vector.tensor_add(x2row[:], x2row[:], hsm[:])
            stats = small.tile([P, NN, 6], f32, tag="stats")
            for c in range(NN):
                nc.vector.bn_stats(stats[:, c, :], x2row[:, ts(c, NCH)])
            mv = small.tile([P, 2], f32, tag="mv")
            nc.vector.bn_aggr(mv[:], stats[:])
            rstd = small.tile([P, 1], f32, tag="rstd")
            nc.scalar.activation(rstd[:], mv[:, 1:2], AF.Sqrt, bias=eps_t[:])
            nc.vector.reciprocal(rstd[:], rstd[:])
            xb = transb.tile([P, H], bf16, tag="xrow")
            nc.vector.tensor_scalar(out=xb[:], in0=x2row[:],
                                    scalar1=mv[:, 0:1], scalar2=rstd[:],
                                    op0=ALU.subtract, op1=ALU.mult)
            prev_xb = (xb, m)
        xb, pm = prev_xb
        transpose_into(bigT, lambda k: xb[:, ts(k, P)], pm, id_bf)

        # ---- syn experts on sharedT (bigT) ----
        fo = 0
        for e in range(3):
            if caps_syn[e]:
                expert_window(wsyn_d[e], bigT, re_syn[e], spans_syn[e],
                              caps_syn[e], bsyn_sb[e], csyn_t, fo, fo)
                fo += caps_syn[e]
        if C_syn:
            nc.gpsimd.dma_start(out_r[:, 0:C_syn, :], out_sb[:, 0:C_syn, :])

        # ---- sem experts on semT ----
        fo = 0
        for e in range(3):
            if caps_sem[e]:
                expert_window(wsem_d[e], semT, re_sem[e], spans_sem[e],
                              caps_sem[e], bsem_sb[e], csem_t, fo,
                              C_syn + ST + fo)
                fo += caps_sem[e]
                nc.gpsimd.dma_start(
                    out_r[:, C_syn + ST + fo - caps_sem[e] : C_syn + ST + fo, :],
                    out_sb[:, C_syn + ST + fo - caps_sem[e] : C_syn + ST + fo, :])

    nc.compile()
    return nc


def _get_program(geom):
    if geom not in _prog_cache:
        _prog_cache[geom] = _build_program(geom)
    return _prog_cache[geom]


# ---------------------------------------------------------------- host glue
def _prepare(inputs):
    """Compute routing, permutations, windows; build per-core in_maps and
    decode metadata. Returns (geom, in_maps, meta)."""
    hs = np.asarray(inputs["hidden_states"], dtype=np.float32)
    adj = np.asarray(inputs["adj_matrix"], dtype=np.float32)
    seq_lengths = np.asarray(inputs["seq_lengths"])
    router_w = np.asarray(inputs["router_w"], dtype=np.float32)
    router_b = np.asarray(inputs["router_b"], dtype=np.float32)
    gcn1_w = np.asarray(inputs["gcn1_w"], dtype=np.float32)
    gcn2_w = np.asarray(inputs["gcn2_w"], dtype=np.float32)
    ln_g = np.asarray(inputs["ln_g"], dtype=np.float32)
    ln_b = np.asarray(inputs["ln_b"], dtype=np.float32)
    syn_w = np.asarray(inputs["syn_w"], dtype=np.float32)
    syn_b = np.asarray(inputs["syn_b"], dtype=np.float32)
    len_short_w = np.asarray(inputs["len_short_w"], dtype=np.float32)
    len_short_b = np.asarray(inputs["len_short_b"], dtype=np.float32)
    len_long_w = np.asarray(inputs["len_long_w"], dtype=np.float32)
    len_long_b = np.asarray(inputs["len_long_b"], dtype=np.float32)
    sem_w = np.asarray(inputs["sem_w"], dtype=np.float32)
    sem_b = np.asarray(inputs["sem_b"], dtype=np.float32)
    cls_w = np.asarray(inputs["cls_w"], dtype=np.float32)
    cls_b = np.asarray(inputs["cls_b"], dtype=np.float32)

    c_syn, syn_i, c_len, c_sem, sem_i, is_short = _route_host(
        hs, router_w, router_b, seq_lengths)

    # fold LN affine into syn weights: LN_plain(x) @ (g*W) + (b@W + bias)
    syn_w_f = (ln_g[None, :, None] * syn_w).astype(np.float32)
    syn_b_f = (syn_b + np.einsum("h,ehd->ed", ln_b, syn_w)).astype(np.float32)

    perm = np.argsort(syn_i, axis=1, kind="stable")          # [B,S]
    syn_i_p = np.take_along_axis(syn_i, perm, axis=1)
    sem_i_p = np.take_along_axis(sem_i, perm, axis=1)
    sem_perm = np.argsort(sem_i_p, axis=1, kind="stable")    # syn-order -> sem-order
    sem_i_s = np.take_along_axis(sem_i_p, sem_perm, axis=1)

    re_syn, caps_syn, spans_syn = _windows(syn_i_p)
    re_sem, caps_sem, spans_sem = _windows(sem_i_s)
    C_syn, C_sem = sum(caps_syn), sum(caps_sem)

    synb_nz = bool(np.any(syn_b_f != 0))
    lenb_nz = bool(np.any(len_short_b != 0) or np.any(len_long_b != 0))
    semb_nz = bool(np.any(sem_b != 0))
    geom = (re_syn, caps_syn, spans_syn, re_sem, caps_sem, spans_sem,
            synb_nz, lenb_nz, semb_nz)

    wg1 = np.ascontiguousarray(gcn1_w.astype(_BF16))
    wg2 = np.ascontiguousarray(gcn2_w.astype(_BF16))
    wsyn = np.ascontiguousarray(syn_w_f.astype(_BF16))
    wlen_s = np.ascontiguousarray(len_short_w.astype(_BF16))
    wlen_l = np.ascontiguousarray(len_long_w.astype(_BF16))
    wsem = np.ascontiguousarray(sem_w.astype(_BF16))
    wcls = np.ascontiguousarray(cls_w.astype(_BF16))

    def win_coef(cvec, idx_sorted_row, re, caps, grp):
        """Per-window masked coefficients, zero-padded to caps*P, duplicated
        over the 2 output classes so the device applies them in one mult."""
        out = np.zeros((max(sum(caps), 1) * P, 2), np.float32)
        off = 0
        for e in range(3):
            w = caps[e] * P
            lo = re[e]
            hi = min(S, lo + w)
            seg = np.where(idx_sorted_row[lo:hi] == e, cvec[lo:hi], 0.0)
            out[off : off + (hi - lo), 0] = seg
            out[off : off + (hi - lo), 1] = seg
            off += w
        return out

    in_maps = []
    meta = []
    for b in range(B):
        p = perm[b]
        sp = sem_perm[b]
        hs_p = hs[b][p]
        adj_p = adj[b][p][:, p]
        deg = np.maximum(adj_p.sum(axis=1, dtype=np.float32), 1e-9)
        adj_n = adj_p / deg[:, None]
        hs_sem = hs_p[sp]
        c_syn_p = c_syn[b][p]
        c_len_p = c_len[b][p]
        c_sem_s = c_sem[b][p][sp]

        m = {
            "hsT": np.ascontiguousarray(hs_p.T.astype(_BF16)),
            "adjT": np.ascontiguousarray(adj_n.T.astype(_BF16)),
            "hs": np.ascontiguousarray(hs_p),
            "semT": np.ascontiguousarray(hs_sem.T.astype(_BF16)),
            "wg1": wg1, "wg2": wg2, "wsyn": wsyn,
            "wlen": wlen_s if is_short[b] else wlen_l,
            "wsem": wsem, "wcls": wcls,
            "csyn": win_coef(c_syn_p, syn_i_p[b], re_syn, caps_syn, "syn"),
            "clen": np.ascontiguousarray(np.stack([c_len_p, c_len_p], axis=1)),
            "csem": win_coef(c_sem_s, sem_i_s[b], re_sem, caps_sem, "sem"),
        }
        if synb_nz:
            m["bsyn"] = syn_b_f
        if lenb_nz:
            m["blen"] = (len_short_b if is_short[b]
                         else len_long_b).reshape(1, H).astype(np.float32)
        if semb_nz:
            m["bsem"] = sem_b.astype(np.float32)
        in_maps.append(m)
        meta.append((p, sp))

    return geom, in_maps, meta, cls_b


def _decode(out_rows, geom, meta_b, cls_b):
    """out_rows: [NT*P, 2] device output for one core -> [S,2] original order."""
    re_syn, caps_syn, re_sem, caps_sem = geom[0], geom[1], geom[3], geom[4]
    C_syn, C_sem = sum(caps_syn), sum(caps_sem)
    p, sp = meta_b
    acc_syn = np.zeros((S, 2), np.float32)   # syn-order accumulation
    off = 0
    for e in range(3):
        w = caps_syn[e] * P
        lo = re_syn[e]
        hi = min(S, lo + w)
        acc_syn[lo:hi] += out_rows[off : off + (hi - lo)]
        off += w
    acc_syn += out_rows[C_syn * P : C_syn * P + S]          # len group
    acc_sem = np.zeros((S, 2), np.float32)   # sem-order
    off = (C_syn + ST) * P
    for e in range(3):
        w = caps_sem[e] * P
        lo = re_sem[e]
        hi = min(S, lo + w)
        acc_sem[lo:hi] += out_rows[off : off + (hi - lo)]
        off += w
    acc_syn[sp] += acc_sem
    res = np.empty((S, 2), np.float32)
    res[p] = acc_syn
    return res + cls_b


def kernel(**inputs):
    from concourse import bass_utils

    geom, in_maps, meta, cls_b = _prepare(inputs)
    nc = _get_program(geom)

    try:
        res = bass_utils.run_bass_kernel_spmd(nc, in_maps, core_ids=list(range(B)))
    except Exception:
        # transient device wedge (NRT_EXEC_UNIT_UNRECOVERABLE) clears on retry
        res = bass_utils.run_bass_kernel_spmd(nc, in_maps, core_ids=list(range(B)))
    globals()["_last_results"] = res
    out = np.stack([_decode(np.asarray(res.results[b]["out"], np.float32),
                            geom, meta[b], cls_b)
                    for b in range(B)]).astype(np.float32)
    return out
